# revision 17
# baseline (speedup 1.0000x reference)
"""VQ codebook assignment (ApplyKmeans) on 8 Trainium2 NeuronCores.

tokens[n] = argmin_k ||x_n - c_k||^2
          = argmax_k (x_n.c_k - Cnorm_k/2)        (||x_n||^2 constant per row)

Data-parallel: x sharded along N across 8 cores, C/Cnorm replicated.

Per core (16384 rows, 128 row-tiles of 128 rows):
  - host pre-tiles x^T so each [128d, 128n] stationary tile is contiguous
    (fp16: halves HBM traffic; PSUM accumulates fp32; ~52/131072 argmin
    flips vs the fp32 reference, rel err ~0.0144)
  - warmup: dep-free matmuls over a memset SBUF tile, cycling the PSUM
    pair-tiles. They execute during the initial DMA wait, ramping the
    PE out of its low p-state, and their start=True writes set every
    PSUM has_written bit - so every real tile uses the ACT-copy bias
    path (ScalarE rewrites the bank to -Cnorm/2, then 8 start=False
    matmuls accumulate on top). No bias matmuls needed.
  - row-tiles are processed in PAIRS sharing one 2-bank PSUM tile
    [128, 2, 512], each half bank-aligned (Tile's dependency tracking
    is bank-granular: an unpadded layout gave half B's matmuls a false
    WAR against half A's MAX8 read, serializing the PE). Per pair:
    one ACT copy writes the bias into both halves (600 elems, halves
    the per-instruction overhead), 16 accumulating matmuls fill the
    halves, DVE runs MAX8 per half, Pool packs [A0..A3,B0..B3] into an
    8-wide key buffer, and ONE FIND_INDEX8 scans the 600-elem pair:
    out[0] = argmax of half A, out[4] = argmax of half B + 300.
    This keeps DVE (~1.84us/pair) under the PE's ~2.03us/pair budget -
    with per-tile FIND the DVE was the co-bottleneck.
  - Pool extracts tokens: even tiles pass through, odd tiles compute
    max(idx,300)-300 (wrap-safe: if half B's max value bit-exactly
    collides with an earlier value in half A - expected ~2 rows per
    full run - the token clamps to 0 instead of wrapping negative).
  - a dummy ACTIVATE right after the const DMA issues pulls the
    1.3us ACT_TABLE_LOAD off the first bias-copy's critical path.
  - startup is DMA-bandwidth-bound (~330GB/s shared by all rings), so
    all startup loads ride the sync ring's single in-order pipe in
    exact first-use order: ct0-1, x tile0, ct2-4, x tile1, ct5-7,
    x tiles 2-3, then the steady groups. The bias rides as a 4.8KB
    fp16 hi/lo pair [2, 600] broadcast by a 2-row PE matmul into a
    warm PSUM tile and ACT-copied once to SBUF - 300KB of startup DMA
    replaced by two 125ns matmuls.
  - group 0 is stored tile-major (xg0) and arrives as 4 single-tile
    DMAs: the first tile's full x^T lands ~256KB after the queue opens,
    so the PE reaches full rate ~5us sooner than with chunk-major
    group-0 loads (where tile 0 needed all 2MB).

Row interleaving: row-tile t holds rows {p*128 + t}, so the token buffer
[p, t] DMAs out contiguously in original row order.

Walrus only lowers one sync wait per instruction; _hoist_excess_waits
moves Tile's extra waits onto same-engine no-ops at the same program
point. Mid-kernel x loads share the sync HWDGE ring (same-ring
transfers complete in order, so prefetch can't starve urgent loads);
constants and token stores ride the scalar ring. Keep KM_HW_LANES=8:
with fewer lanes the scalar ring's startup DMAs serialize behind each
other's transfers (lane-reuse WAW), costing ~6us.
"""

import os
import sys

import numpy as np

if "/opt/trn_rl_repo" not in sys.path:
    sys.path.insert(0, "/opt/trn_rl_repo")

import concourse.bass as bass
import concourse.mybir as mybir
import concourse.tile_sem_assignment as _tsa
from concourse.bass_utils import run_bass_kernel_spmd
from concourse.tile import TileContext

_tsa.NUM_HWDGE_SEMS = int(os.environ.get("KM_HW_LANES", "8"))

# Give each HWDGE ring (SP-issued vs ACT-issued DMAs) a disjoint pool of
# completion lanes. Tile's global round-robin otherwise interleaves the
# two rings onto shared lanes, and the lane-order WAW waits then falsely
# serialize one ring behind the other.
_orig_assign_tick = _tsa.TileClockTick._assign_tick


def _assign_tick_lanepools(self, inst):
    try:
        if isinstance(inst, _tsa.DMAInst) and inst.engine != mybir.EngineType.Pool:
            if not hasattr(self, "_lane_ctr"):
                self._lane_ctr = {}
            eng = inst.engine
            n = _tsa.NUM_HWDGE_SEMS
            half = max(1, n // 2)
            pool = (
                list(range(0, half))
                if eng == mybir.EngineType.Activation
                else list(range(half, n))
            )
            c = self._lane_ctr.get(eng, 0)
            self.next_hw_dma_idx = pool[c % len(pool)]
            self._lane_ctr[eng] = c + 1
    except Exception:
        pass
    return _orig_assign_tick(self, inst)


_tsa.TileClockTick._assign_tick = _assign_tick_lanepools

P = 128
D = 1024
K = 300
NCORES = 8
ROWS = 16384            # rows per core
TILES = ROWS // P       # 128 row-tiles per core
PAIRS = TILES // 2      # 64 PSUM pair-tiles per core
HWC = 512               # fp32 cols per PSUM half (2KB = one bank)
GROUPS = 32             # DMA groups per core (1 group = 1 MB fp16)
TPG = TILES // GROUPS   # 8 row-tiles per group
DCH = D // P            # 8 contraction chunks

F16 = mybir.dt.float16
F32 = mybir.dt.float32
I32 = mybir.dt.int32
U32 = mybir.dt.uint32

# Set by kernel() so test.py can read profiling info.
LAST_RESULT = None


def _ensure_ntff_hook():
    """Install antenv.axon_hooks shim so trace=True works under axon."""
    try:
        from antenv.axon_hooks import get_axon_ntff_profile_hook  # noqa: F401

        return
    except ImportError:
        pass
    import types

    import antenv

    try:
        from trn_agent_boot.trn_boot import _ntff_profile_via_ctypes
    except ImportError:
        return
    mod = types.ModuleType("antenv.axon_hooks")
    _hook = [None]
    mod.set_axon_ntff_profile_hook = lambda h: _hook.__setitem__(0, h)
    mod.get_axon_ntff_profile_hook = lambda: _hook[0]
    sys.modules["antenv.axon_hooks"] = mod
    antenv.axon_hooks = mod
    so = "/opt/axon/libaxon_pjrt.so"
    if os.path.exists(so):
        mod.set_axon_ntff_profile_hook(_ntff_profile_via_ctypes(so))


# Token flush boundaries (exclusive tile index, all even): 16-tile blocks
# through t=112, then 8/6/2 so the final CAST+DMA is tiny and the tail
# is short.
FLUSH = [16, 32, 48, 64, 80, 96, 112, 120, 126, 128]


def _max_index_nd(nc, out, in_max, in_values):
    """max_index with a multi-dim in_values AP (bass asserts 2D, the DVE
    ISA takes general APs; indices are in AP traversal order)."""
    eng = nc.vector
    return eng.add_instruction(
        mybir.InstMaxIndex(
            name=nc.get_next_instruction_name(),
            ins=[eng.lower_ap(in_max), eng.lower_ap(in_values)],
            outs=[eng.lower_ap(out)],
        )
    )


def _emit_flush(nc, out, tokbuf, idxbuf, bounds):
    """Pool extracts tokens for tiles [s, e) and the scalar ring DMAs
    them out. Even tiles pass through; odd tiles un-offset wrap-safely:
    max(idx, 300) - 300 clamps the rare cross-half value collision to
    token 0 instead of wrapping negative."""
    s, e = bounds
    s2, e2 = s // 2, e // 2
    nc.gpsimd.tensor_copy(out=tokbuf[:, s:e:2], in_=idxbuf[:, s2:e2, 0])
    nc.gpsimd.tensor_scalar(
        tokbuf[:, s + 1 : e : 2],
        idxbuf[:, s2:e2, 4],
        300,
        300,
        mybir.AluOpType.max,
        mybir.AluOpType.subtract,
    )
    nc.scalar.dma_start(out=out[:, s:e], in_=tokbuf[:, s:e])


def build_nc() -> bass.Bass:
    n_warm = int(os.environ.get("KM_WARM", "10"))
    spread0 = bool(int(os.environ.get("KM_SPREAD0", "1")))
    table_hoist = bool(int(os.environ.get("KM_TABLE_HOIST", "1")))

    nc = bass.Bass()

    xg = nc.declare_dram_parameter("xg", [GROUPS, P, DCH * TPG * P], F16, isOutput=False)
    xg0 = nc.declare_dram_parameter("xg0", [TPG, P, DCH * P], F16, isOutput=False)
    cons = nc.declare_dram_parameter("cons", [P, DCH * K], F16, isOutput=False)
    bias2 = nc.declare_dram_parameter("bias2", [2, 2 * K], F16, isOutput=False)
    out = nc.declare_dram_parameter("out", [P, TILES], I32, isOutput=True)

    with TileContext(nc) as tc:
        with (
            tc.tile_pool(name="const", bufs=1) as constp,
            tc.tile_pool(name="warm", bufs=1) as warmp,
            tc.tile_pool(name="xp0", bufs=TPG) as xp0,
            tc.tile_pool(name="xp", bufs=5) as xp,
            tc.tile_pool(name="mx", bufs=8) as mxp,
            tc.tile_pool(name="mx8", bufs=4) as mx8p,
            tc.tile_pool(name="psum", bufs=4, space="PSUM") as psp,
            tc.tile_pool(name="outp", bufs=1) as outp,
        ):
            # tiny bias load on the scalar ring; everything else rides
            # the sync ring's in-order pipe in first-use order
            b2t = constp.tile([2, 2 * K], F16)
            nc.scalar.dma_start(out=b2t[:], in_=bias2[:])
            cons_t = constp.tile([P, DCH * K], F16)
            ctiles = [cons_t[:, j * K : (j + 1) * K] for j in range(DCH)]

            warm = warmp.tile([P, K], F16)
            nc.gpsimd.memset(warm[:], 0.0)
            ones2 = warmp.tile([2, P], F16)
            nc.gpsimd.memset(ones2[:], 1.0)
            if table_hoist:
                # first InstActivation triggers the 1.3us ACT_TABLE_LOAD;
                # issue a dummy now so it overlaps the const DMAs instead
                # of gating the first bias copy
                tdum = warmp.tile([P, 1], F32)
                nc.scalar.copy(out=tdum[:], in_=warm[:, :1])

            # PE warmup: dep-free matmuls over a memset SBUF tile,
            # cycling the PSUM pair-tiles. They run during the startup
            # DMA wait (ramping the PE p-state) and their start=True
            # writes set every half-bank's has_written bits. Extra warms
            # (w8+) go to wtiles[2] so pairs 0/1 aren't delayed.
            wtiles = [psp.tile([P, 2, HWC], F32, name="ps") for _ in range(4)]
            worder = [0, 0, 1, 1, 2, 2, 3, 3] + [2] * 8
            for w in range(n_warm):
                pr, half = worder[w], w % 2
                nc.tensor.matmul(
                    wtiles[pr][:, half, :K],
                    lhsT=warm[:, :P], rhs=warm[:, :K],
                    start=True, stop=True,
                )

            # bias broadcast: ones2^T @ [bh; bl] accumulates bh+bl in
            # fp32 into wtiles[3], ACT copies it once to SBUF
            bft = constp.tile([P, 2, K], F32)
            for h in range(2):
                nc.tensor.matmul(
                    wtiles[3][:, h, :K],
                    lhsT=ones2[:], rhs=b2t[:, h * K : (h + 1) * K],
                    start=True, stop=True,
                )
            nc.scalar.copy(out=bft[:], in_=wtiles[3][:, :, :K])

            # sync-ring in-order pipe: consts and group-0 x tiles in
            # exact first-use order
            xch0 = []

            def _load_x0(tl):
                cbuf = xp0.tile([P, DCH, P], F16, name="xtile")
                nc.sync.dma_start(
                    out=cbuf[:], in_=xg0[tl].rearrange("p (j q) -> p j q", j=DCH)
                )
                xch0.append(cbuf)

            nc.sync.dma_start(out=cons_t[:, : 2 * K], in_=cons[:, : 2 * K])
            _load_x0(0)
            nc.sync.dma_start(out=cons_t[:, 2 * K : 5 * K], in_=cons[:, 2 * K : 5 * K])
            _load_x0(1)
            nc.sync.dma_start(out=cons_t[:, 5 * K :], in_=cons[:, 5 * K :])
            _load_x0(2)
            _load_x0(3)

            idxbuf = outp.tile([P, PAIRS, 8], U32)
            tokbuf = outp.tile([P, TILES], I32)
            pending_flush = None

            for g in range(GROUPS):
                if g == 0:
                    chunk = lambda j, tl: xch0[tl][:, j, :]
                else:
                    # all steady-state x loads share the sync ring:
                    # same-ring transfers serialize per DMA-engine FIFO,
                    # so prefetch can't steal bandwidth from earlier
                    # (more urgent) loads
                    xbuf = xp.tile([P, DCH, TPG, P], F16, name="xgrp")
                    nc.sync.dma_start(
                        out=xbuf[:],
                        in_=xg[g].rearrange("p (j t q) -> p j t q", j=DCH, t=TPG),
                    )
                    chunk = lambda j, tl, xbuf=xbuf: xbuf[:, j, tl, :]
                for pl in range(TPG // 2):
                    pr = g * (TPG // 2) + pl
                    flush_now, pending_flush = pending_flush, None
                    # the first 4 pairs reuse the warmup tiles directly:
                    # same logical tile, so warm -> bias -> matmul is a
                    # plain same-tile WAW chain
                    if pr < 4:
                        ps = wtiles[pr]
                    else:
                        ps = psp.tile([P, 2, HWC], F32, name="ps")
                    # has_written bits persist from this tile's previous
                    # occupant (warmup or prior pair); ScalarE resets the
                    # values to the bias and the start=False matmuls
                    # accumulate on top - one ACTIVATE covers both halves
                    nc.scalar.copy(out=ps[:, :, :K], in_=bft[:])
                    mxt = mxp.tile([P, 2, 8], F32)
                    for half in range(2):
                        tl = 2 * pl + half
                        for j in range(DCH):
                            nc.tensor.matmul(
                                ps[:, half, :K],
                                lhsT=chunk(j, tl),
                                rhs=ctiles[j][:],
                                start=False,
                                stop=(j == DCH - 1),
                                skip_group_check=True,
                            )
                        nc.vector.max(out=mxt[:, half, :], in_=ps[:, half, :K])
                    # Pool packs the FIND keys [A0..A3, B0..B3]; only
                    # lanes 0 (maxA) and 4 (maxB) are consumed, the rest
                    # are benign real values that keep every lane written
                    mx8 = mx8p.tile([P, 8], F32)
                    nc.gpsimd.tensor_copy(out=mx8[:], in_=mxt[:, :, 0:4])
                    # one FIND over the 600-elem pair: out[0] = idx of
                    # maxA (in [0,300)), out[4] = idx of maxB + 300
                    _max_index_nd(
                        nc, out=idxbuf[:, pr, :], in_max=mx8[:],
                        in_values=ps[:, :, :K],
                    )
                    if flush_now is not None:
                        _emit_flush(nc, out, tokbuf, idxbuf, flush_now)
                    # queue this pair's flush for emission one pair later:
                    # emitting it here would park the token-DMA issue in the
                    # scalar queue ahead of the NEXT pair's bias ACTIVATE,
                    # and the in-order queue then stalls the PE on FIND
                    t = 2 * pr + 1
                    if (t + 1) in FLUSH:
                        s = FLUSH[FLUSH.index(t + 1) - 1] if (t + 1) != FLUSH[0] else 0
                        if t + 1 == TILES:
                            _emit_flush(nc, out, tokbuf, idxbuf, (s, t + 1))
                        else:
                            pending_flush = (s, t + 1)

    _hoist_excess_waits(nc)
    return nc


def _hoist_excess_waits(nc: bass.Bass, max_waits: int = 1):
    """Hoist excess sync waits onto no-op drains inserted just before.

    Walrus's codegen caps embedded sync waits per instruction (1 for
    DIRECT2D DMAs and CTRL ops), but Tile can attach several (slot-reuse
    WAR + lane WAW, or the kernel-tail drain waiting on every proc).
    A same-engine drain immediately before the instruction blocks the
    sequencer at the same program point, so semantics are unchanged.
    """
    n = 0
    for f in nc.m.functions:
        for blk in f.blocks:
            insts = blk.instructions
            i = 0
            while i < len(insts):
                inst = insts[i]
                si = inst.sync_info
                if si and si.on_wait and len(si.on_wait) > max_waits:
                    waits = list(si.on_wait)
                    si.on_wait = waits[-max_waits:]
                    inst.sync_info = si
                    pre = []
                    for j in range(0, len(waits) - max_waits, max_waits):
                        nd = mybir.InstNoOp(name=f"I-wsplit{n}", ins=[], outs=[])
                        n += 1
                        nd.engine = inst.engine
                        nsi = type(si)(
                            on_wait=waits[j : j + max_waits], on_update=[]
                        )
                        nd.sync_info = nsi
                        try:
                            nc.register_instruction(nd, overwrite=True)
                        except Exception:
                            pass
                        pre.append(nd)
                    for k, nd in enumerate(pre):
                        insts.insert(i + k, nd)
                    i += len(pre)
                i += 1


def make_in_maps(x: np.ndarray, C: np.ndarray, Cnorm: np.ndarray):
    x16 = x.astype(np.float16)
    C16 = C.astype(np.float16).reshape(DCH, P, K)

    cons = np.ascontiguousarray(C16.transpose(1, 0, 2).reshape(P, DCH * K))
    b1 = (-0.5 * Cnorm.reshape(K)).astype(np.float32)
    bh = b1.astype(np.float16)
    bl = (b1 - bh.astype(np.float32)).astype(np.float16)
    bias2 = np.stack([np.concatenate([bh, bh]), np.concatenate([bl, bl])])

    in_maps = []
    for c in range(NCORES):
        xs = x16[c * ROWS : (c + 1) * ROWS]
        # row r = p*128 + g*TPG + tl ; col = j*128 + pd
        xr = xs.reshape(P, GROUPS, TPG, DCH, P)          # [p, g, tl, j, pd]
        xgc = np.ascontiguousarray(xr.transpose(1, 4, 3, 2, 0))  # [g, pd, j, tl, p]
        xg0 = np.ascontiguousarray(xr[:, 0].transpose(1, 3, 2, 0))  # [tl, pd, j, p]
        in_maps.append(
            {
                "xg": xgc.reshape(GROUPS, P, DCH * TPG * P),
                "xg0": xg0.reshape(TPG, P, DCH * P),
                "cons": cons,
                "bias2": bias2,
            }
        )
    return in_maps


_NC_CACHE = {}


def kernel(x, C, Cnorm, b, t):
    global LAST_RESULT
    x = np.asarray(x)
    C = np.asarray(C)
    Cnorm = np.asarray(Cnorm)

    key = 0
    if key not in _NC_CACHE:
        _NC_CACHE[key] = build_nc()
    nc = _NC_CACHE[key]

    in_maps = make_in_maps(x, C, Cnorm)
    trace = bool(int(os.environ.get("KM_TRACE", "0")))
    if trace:
        _ensure_ntff_hook()
    res = run_bass_kernel_spmd(
        nc, in_maps, core_ids=list(range(NCORES)), trace=trace
    )
    LAST_RESULT = res

    shards = [res.results[c]["out"].reshape(-1) for c in range(NCORES)]
    tokens = np.concatenate(shards).astype(np.int32)
    return tokens.reshape(int(b), int(t))


# revision 18
# speedup vs baseline: 1.0053x; 1.0053x over previous
"""VQ codebook assignment (ApplyKmeans) on 8 Trainium2 NeuronCores.

tokens[n] = argmin_k ||x_n - c_k||^2
          = argmax_k (x_n.c_k - Cnorm_k/2)        (||x_n||^2 constant per row)

Data-parallel: x sharded along N across 8 cores, C/Cnorm replicated.

Per core (16384 rows, 128 row-tiles of 128 rows):
  - host pre-tiles x^T so each [128d, 128n] stationary tile is contiguous
    (fp16: halves HBM traffic; PSUM accumulates fp32; ~52/131072 argmin
    flips vs the fp32 reference, rel err ~0.0144)
  - warmup: dep-free matmuls over a memset SBUF tile, cycling the PSUM
    pair-tiles. They execute during the initial DMA wait, ramping the
    PE out of its low p-state, and their start=True writes set every
    PSUM has_written bit - so every real tile uses the ACT-copy bias
    path (ScalarE rewrites the bank to -Cnorm/2, then 8 start=False
    matmuls accumulate on top). No bias matmuls needed.
  - row-tiles are processed in PAIRS sharing one 2-bank PSUM tile
    [128, 2, 512], each half bank-aligned (Tile's dependency tracking
    is bank-granular: an unpadded layout gave half B's matmuls a false
    WAR against half A's MAX8 read, serializing the PE). Per pair:
    one ACT copy writes the bias into both halves (600 elems, halves
    the per-instruction overhead), 16 accumulating matmuls fill the
    halves, DVE runs MAX8 per half, Pool packs [A0..A3,B0..B3] into an
    8-wide key buffer, and ONE FIND_INDEX8 scans the 600-elem pair:
    out[0] = argmax of half A, out[4] = argmax of half B + 300.
    This keeps DVE (~1.84us/pair) under the PE's ~2.03us/pair budget -
    with per-tile FIND the DVE was the co-bottleneck.
  - Pool extracts tokens: even tiles pass through, odd tiles compute
    max(idx,300)-300 (wrap-safe: if half B's max value bit-exactly
    collides with an earlier value in half A - expected ~2 rows per
    full run - the token clamps to 0 instead of wrapping negative).
  - a dummy ACTIVATE right after the const DMA issues pulls the
    1.3us ACT_TABLE_LOAD off the first bias-copy's critical path.
  - startup is DMA-bandwidth-bound (~330GB/s shared by all rings), so
    all startup loads ride the sync ring's single in-order pipe in
    exact first-use order: ct0-1, x tile0, ct2-4, x tile1, ct5-7,
    x tiles 2-3, then the steady groups. The bias rides as a 4.8KB
    fp16 hi/lo pair [2, 600] broadcast by a 2-row PE matmul into a
    warm PSUM tile and ACT-copied once to SBUF - 300KB of startup DMA
    replaced by two 125ns matmuls.
  - group 0 is stored tile-major (xg0) and arrives as 4 single-tile
    DMAs: the first tile's full x^T lands ~256KB after the queue opens,
    so the PE reaches full rate ~5us sooner than with chunk-major
    group-0 loads (where tile 0 needed all 2MB).

Row interleaving: row-tile t holds rows {p*128 + t}, so the token buffer
[p, t] DMAs out contiguously in original row order.

Walrus only lowers one sync wait per instruction; _hoist_excess_waits
moves Tile's extra waits onto same-engine no-ops at the same program
point. Mid-kernel x loads share the sync HWDGE ring (same-ring
transfers complete in order, so prefetch can't starve urgent loads);
constants and token stores ride the scalar ring. Keep KM_HW_LANES=8:
with fewer lanes the scalar ring's startup DMAs serialize behind each
other's transfers (lane-reuse WAW), costing ~6us.
"""

import os
import sys

import numpy as np

if "/opt/trn_rl_repo" not in sys.path:
    sys.path.insert(0, "/opt/trn_rl_repo")

import concourse.bass as bass
import concourse.mybir as mybir
import concourse.tile_sem_assignment as _tsa
from concourse.bass_utils import run_bass_kernel_spmd
from concourse.tile import TileContext

_tsa.NUM_HWDGE_SEMS = int(os.environ.get("KM_HW_LANES", "8"))

# Give each HWDGE ring (SP-issued vs ACT-issued DMAs) a disjoint pool of
# completion lanes. Tile's global round-robin otherwise interleaves the
# two rings onto shared lanes, and the lane-order WAW waits then falsely
# serialize one ring behind the other.
_orig_assign_tick = _tsa.TileClockTick._assign_tick


def _assign_tick_lanepools(self, inst):
    try:
        if isinstance(inst, _tsa.DMAInst) and inst.engine != mybir.EngineType.Pool:
            if not hasattr(self, "_lane_ctr"):
                self._lane_ctr = {}
            eng = inst.engine
            n = _tsa.NUM_HWDGE_SEMS
            half = max(1, n // 2)
            pool = (
                list(range(0, half))
                if eng == mybir.EngineType.Activation
                else list(range(half, n))
            )
            c = self._lane_ctr.get(eng, 0)
            self.next_hw_dma_idx = pool[c % len(pool)]
            self._lane_ctr[eng] = c + 1
    except Exception:
        pass
    return _orig_assign_tick(self, inst)


_tsa.TileClockTick._assign_tick = _assign_tick_lanepools

P = 128
D = 1024
K = 300
NCORES = 8
ROWS = 16384            # rows per core
TILES = ROWS // P       # 128 row-tiles per core
PAIRS = TILES // 2      # 64 PSUM pair-tiles per core
HWC = 512               # fp32 cols per PSUM half (2KB = one bank)
GROUPS = 32             # DMA groups per core (1 group = 1 MB fp16)
TPG = TILES // GROUPS   # 8 row-tiles per group
DCH = D // P            # 8 contraction chunks

F16 = mybir.dt.float16
F32 = mybir.dt.float32
I32 = mybir.dt.int32
U32 = mybir.dt.uint32

# Set by kernel() so test.py can read profiling info.
LAST_RESULT = None


def _ensure_ntff_hook():
    """Install antenv.axon_hooks shim so trace=True works under axon."""
    try:
        from antenv.axon_hooks import get_axon_ntff_profile_hook  # noqa: F401

        return
    except ImportError:
        pass
    import types

    import antenv

    try:
        from trn_agent_boot.trn_boot import _ntff_profile_via_ctypes
    except ImportError:
        return
    mod = types.ModuleType("antenv.axon_hooks")
    _hook = [None]
    mod.set_axon_ntff_profile_hook = lambda h: _hook.__setitem__(0, h)
    mod.get_axon_ntff_profile_hook = lambda: _hook[0]
    sys.modules["antenv.axon_hooks"] = mod
    antenv.axon_hooks = mod
    so = "/opt/axon/libaxon_pjrt.so"
    if os.path.exists(so):
        mod.set_axon_ntff_profile_hook(_ntff_profile_via_ctypes(so))


# Token flush boundaries (exclusive tile index, all even): 16-tile blocks
# through t=112, then 8/6/2 so the final CAST+DMA is tiny and the tail
# is short.
FLUSH = [16, 32, 48, 64, 80, 96, 112, 120, 126, 128]


def _max_index_nd(nc, out, in_max, in_values):
    """max_index with a multi-dim in_values AP (bass asserts 2D, the DVE
    ISA takes general APs; indices are in AP traversal order)."""
    eng = nc.vector
    return eng.add_instruction(
        mybir.InstMaxIndex(
            name=nc.get_next_instruction_name(),
            ins=[eng.lower_ap(in_max), eng.lower_ap(in_values)],
            outs=[eng.lower_ap(out)],
        )
    )


def _emit_flush(nc, out, tokbuf, idxbuf, bounds):
    """Pool extracts tokens for tiles [s, e) and the scalar ring DMAs
    them out. Even tiles pass through; odd tiles un-offset wrap-safely:
    max(idx, 300) - 300 clamps the rare cross-half value collision to
    token 0 instead of wrapping negative."""
    s, e = bounds
    s2, e2 = s // 2, e // 2
    nc.gpsimd.tensor_copy(out=tokbuf[:, s:e:2], in_=idxbuf[:, s2:e2, 0])
    nc.gpsimd.tensor_scalar(
        tokbuf[:, s + 1 : e : 2],
        idxbuf[:, s2:e2, 4],
        300,
        300,
        mybir.AluOpType.max,
        mybir.AluOpType.subtract,
    )
    nc.scalar.dma_start(out=out[:, s:e], in_=tokbuf[:, s:e])


def build_nc() -> bass.Bass:
    n_warm = int(os.environ.get("KM_WARM", "8"))
    spread0 = bool(int(os.environ.get("KM_SPREAD0", "1")))
    table_hoist = bool(int(os.environ.get("KM_TABLE_HOIST", "1")))

    nc = bass.Bass()

    xg = nc.declare_dram_parameter("xg", [GROUPS, P, DCH * TPG * P], F16, isOutput=False)
    xg0 = nc.declare_dram_parameter("xg0", [TPG, P, DCH * P], F16, isOutput=False)
    cons = nc.declare_dram_parameter("cons", [P, DCH * K], F16, isOutput=False)
    bias2 = nc.declare_dram_parameter("bias2", [2, 2 * K], F16, isOutput=False)
    out = nc.declare_dram_parameter("out", [P, TILES], I32, isOutput=True)

    with TileContext(nc) as tc:
        with (
            tc.tile_pool(name="const", bufs=1) as constp,
            tc.tile_pool(name="warm", bufs=1) as warmp,
            tc.tile_pool(name="xp0", bufs=TPG) as xp0,
            tc.tile_pool(name="xp", bufs=5) as xp,
            tc.tile_pool(name="mx", bufs=8) as mxp,
            tc.tile_pool(name="mx8", bufs=4) as mx8p,
            tc.tile_pool(name="psum", bufs=4, space="PSUM") as psp,
            tc.tile_pool(name="outp", bufs=1) as outp,
        ):
            # tiny bias load on the scalar ring; everything else rides
            # the sync ring's in-order pipe in first-use order
            b2t = constp.tile([2, 2 * K], F16)
            nc.scalar.dma_start(out=b2t[:], in_=bias2[:])
            cons_t = constp.tile([P, DCH * K], F16)
            ctiles = [cons_t[:, j * K : (j + 1) * K] for j in range(DCH)]

            warm = warmp.tile([P, K], F16)
            nc.gpsimd.memset(warm[:], 0.0)
            ones2 = warmp.tile([2, P], F16)
            nc.gpsimd.memset(ones2[:], 1.0)
            if table_hoist:
                # first InstActivation triggers the 1.3us ACT_TABLE_LOAD;
                # issue a dummy now so it overlaps the const DMAs instead
                # of gating the first bias copy
                tdum = warmp.tile([P, 1], F32)
                nc.scalar.copy(out=tdum[:], in_=warm[:, :1])

            # PE warmup: dep-free matmuls over a memset SBUF tile into
            # the first pair tiles (overwritten by the bias broadcasts
            # below). They run during the startup DMA wait, ramping the
            # PE p-state so the first real matmuls start at full clock.
            wtiles = [psp.tile([P, 2, HWC], F32, name="ps") for _ in range(4)]
            for w in range(n_warm):
                nc.tensor.matmul(
                    wtiles[(w // 2) % 4][:, w % 2, :K],
                    lhsT=warm[:, :P], rhs=warm[:, :K],
                    start=True, stop=True,
                )

            bft = constp.tile([P, 2, K], F32)

            # sync-ring in-order pipe: group-0 x tiles and consts in
            # exact first-use order
            xch0 = []

            def _load_x0(tl):
                cbuf = xp0.tile([P, DCH, P], F16, name="xtile")
                nc.sync.dma_start(
                    out=cbuf[:], in_=xg0[tl].rearrange("p (j q) -> p j q", j=DCH)
                )
                xch0.append(cbuf)

            _load_x0(0)
            nc.sync.dma_start(out=cons_t[:, : 2 * K], in_=cons[:, : 2 * K])
            _load_x0(1)
            nc.sync.dma_start(out=cons_t[:, 2 * K : 5 * K], in_=cons[:, 2 * K : 5 * K])
            _load_x0(2)
            nc.sync.dma_start(out=cons_t[:, 5 * K :], in_=cons[:, 5 * K :])
            _load_x0(3)

            idxbuf = outp.tile([P, PAIRS, 8], U32)
            tokbuf = outp.tile([P, TILES], I32)
            pending_flush = None

            for g in range(GROUPS):
                if g == 0:
                    chunk = lambda j, tl: xch0[tl][:, j, :]
                else:
                    # all steady-state x loads share the sync ring:
                    # same-ring transfers serialize per DMA-engine FIFO,
                    # so prefetch can't steal bandwidth from earlier
                    # (more urgent) loads
                    xbuf = xp.tile([P, DCH, TPG, P], F16, name="xgrp")
                    nc.sync.dma_start(
                        out=xbuf[:],
                        in_=xg[g].rearrange("p (j t q) -> p j t q", j=DCH, t=TPG),
                    )
                    chunk = lambda j, tl, xbuf=xbuf: xbuf[:, j, tl, :]
                for pl in range(TPG // 2):
                    pr = g * (TPG // 2) + pl
                    flush_now, pending_flush = pending_flush, None
                    if pr < 4:
                        # pairs 0-3: bias arrives via two PE broadcast
                        # matmuls (ones2^T @ [bh; bl] accumulates bh+bl
                        # in fp32) - start=True also sets the half-bank's
                        # has_written bits, and the short PE chain beats
                        # waiting for an ACT round-trip at startup
                        ps = wtiles[pr]
                        for h in range(2):
                            nc.tensor.matmul(
                                ps[:, h, :K],
                                lhsT=ones2[:], rhs=b2t[:, h * K : (h + 1) * K],
                                start=True, stop=True,
                            )
                        if pr == 2:
                            # snapshot the bias into SBUF for pairs 4+
                            # (before this pair's matmuls clobber it)
                            nc.scalar.copy(out=bft[:], in_=ps[:, :, :K])
                    else:
                        ps = psp.tile([P, 2, HWC], F32, name="ps")
                        # has_written bits persist from this bank's prior
                        # occupant; ScalarE resets the values to the bias
                        # and the start=False matmuls accumulate on top
                        nc.scalar.copy(out=ps[:, :, :K], in_=bft[:])
                    mxt = mxp.tile([P, 2, 8], F32)
                    for half in range(2):
                        tl = 2 * pl + half
                        for j in range(DCH):
                            nc.tensor.matmul(
                                ps[:, half, :K],
                                lhsT=chunk(j, tl),
                                rhs=ctiles[j][:],
                                start=False,
                                stop=(j == DCH - 1),
                                skip_group_check=True,
                            )
                        nc.vector.max(out=mxt[:, half, :], in_=ps[:, half, :K])
                    # Pool packs the FIND keys [A0..A3, B0..B3]; only
                    # lanes 0 (maxA) and 4 (maxB) are consumed, the rest
                    # are benign real values that keep every lane written
                    mx8 = mx8p.tile([P, 8], F32)
                    nc.gpsimd.tensor_copy(out=mx8[:], in_=mxt[:, :, 0:4])
                    # one FIND over the 600-elem pair: out[0] = idx of
                    # maxA (in [0,300)), out[4] = idx of maxB + 300
                    _max_index_nd(
                        nc, out=idxbuf[:, pr, :], in_max=mx8[:],
                        in_values=ps[:, :, :K],
                    )
                    if flush_now is not None:
                        _emit_flush(nc, out, tokbuf, idxbuf, flush_now)
                    # queue this pair's flush for emission one pair later:
                    # emitting it here would park the token-DMA issue in the
                    # scalar queue ahead of the NEXT pair's bias ACTIVATE,
                    # and the in-order queue then stalls the PE on FIND
                    t = 2 * pr + 1
                    if (t + 1) in FLUSH:
                        s = FLUSH[FLUSH.index(t + 1) - 1] if (t + 1) != FLUSH[0] else 0
                        if t + 1 == TILES:
                            _emit_flush(nc, out, tokbuf, idxbuf, (s, t + 1))
                        else:
                            pending_flush = (s, t + 1)

    _hoist_excess_waits(nc)
    return nc


def _hoist_excess_waits(nc: bass.Bass, max_waits: int = 1):
    """Hoist excess sync waits onto no-op drains inserted just before.

    Walrus's codegen caps embedded sync waits per instruction (1 for
    DIRECT2D DMAs and CTRL ops), but Tile can attach several (slot-reuse
    WAR + lane WAW, or the kernel-tail drain waiting on every proc).
    A same-engine drain immediately before the instruction blocks the
    sequencer at the same program point, so semantics are unchanged.
    """
    n = 0
    for f in nc.m.functions:
        for blk in f.blocks:
            insts = blk.instructions
            i = 0
            while i < len(insts):
                inst = insts[i]
                si = inst.sync_info
                if si and si.on_wait and len(si.on_wait) > max_waits:
                    waits = list(si.on_wait)
                    si.on_wait = waits[-max_waits:]
                    inst.sync_info = si
                    pre = []
                    for j in range(0, len(waits) - max_waits, max_waits):
                        nd = mybir.InstNoOp(name=f"I-wsplit{n}", ins=[], outs=[])
                        n += 1
                        nd.engine = inst.engine
                        nsi = type(si)(
                            on_wait=waits[j : j + max_waits], on_update=[]
                        )
                        nd.sync_info = nsi
                        try:
                            nc.register_instruction(nd, overwrite=True)
                        except Exception:
                            pass
                        pre.append(nd)
                    for k, nd in enumerate(pre):
                        insts.insert(i + k, nd)
                    i += len(pre)
                i += 1


def make_in_maps(x: np.ndarray, C: np.ndarray, Cnorm: np.ndarray):
    x16 = x.astype(np.float16)
    C16 = C.astype(np.float16).reshape(DCH, P, K)

    cons = np.ascontiguousarray(C16.transpose(1, 0, 2).reshape(P, DCH * K))
    b1 = (-0.5 * Cnorm.reshape(K)).astype(np.float32)
    bh = b1.astype(np.float16)
    bl = (b1 - bh.astype(np.float32)).astype(np.float16)
    bias2 = np.stack([np.concatenate([bh, bh]), np.concatenate([bl, bl])])

    in_maps = []
    for c in range(NCORES):
        xs = x16[c * ROWS : (c + 1) * ROWS]
        # row r = p*128 + g*TPG + tl ; col = j*128 + pd
        xr = xs.reshape(P, GROUPS, TPG, DCH, P)          # [p, g, tl, j, pd]
        xgc = np.ascontiguousarray(xr.transpose(1, 4, 3, 2, 0))  # [g, pd, j, tl, p]
        xg0 = np.ascontiguousarray(xr[:, 0].transpose(1, 3, 2, 0))  # [tl, pd, j, p]
        in_maps.append(
            {
                "xg": xgc.reshape(GROUPS, P, DCH * TPG * P),
                "xg0": xg0.reshape(TPG, P, DCH * P),
                "cons": cons,
                "bias2": bias2,
            }
        )
    return in_maps


_NC_CACHE = {}


def kernel(x, C, Cnorm, b, t):
    global LAST_RESULT
    x = np.asarray(x)
    C = np.asarray(C)
    Cnorm = np.asarray(Cnorm)

    key = 0
    if key not in _NC_CACHE:
        _NC_CACHE[key] = build_nc()
    nc = _NC_CACHE[key]

    in_maps = make_in_maps(x, C, Cnorm)
    trace = bool(int(os.environ.get("KM_TRACE", "0")))
    if trace:
        _ensure_ntff_hook()
    res = run_bass_kernel_spmd(
        nc, in_maps, core_ids=list(range(NCORES)), trace=trace
    )
    LAST_RESULT = res

    shards = [res.results[c]["out"].reshape(-1) for c in range(NCORES)]
    tokens = np.concatenate(shards).astype(np.int32)
    return tokens.reshape(int(b), int(t))


# revision 19
# speedup vs baseline: 1.0093x; 1.0040x over previous
"""VQ codebook assignment (ApplyKmeans) on 8 Trainium2 NeuronCores.

tokens[n] = argmin_k ||x_n - c_k||^2
          = argmax_k (x_n.c_k - Cnorm_k/2)        (||x_n||^2 constant per row)

Data-parallel: x sharded along N across 8 cores, C/Cnorm replicated.

Per core (16384 rows, 128 row-tiles of 128 rows):
  - host pre-tiles x^T so each [128d, 128n] stationary tile is contiguous
    (fp16: halves HBM traffic; PSUM accumulates fp32; ~52/131072 argmin
    flips vs the fp32 reference, rel err ~0.0144)
  - warmup: dep-free matmuls over a memset SBUF tile, cycling the PSUM
    pair-tiles. They execute during the initial DMA wait, ramping the
    PE out of its low p-state, and their start=True writes set every
    PSUM has_written bit - so every real tile uses the ACT-copy bias
    path (ScalarE rewrites the bank to -Cnorm/2, then 8 start=False
    matmuls accumulate on top). No bias matmuls needed.
  - row-tiles are processed in PAIRS sharing one 2-bank PSUM tile
    [128, 2, 512], each half bank-aligned (Tile's dependency tracking
    is bank-granular: an unpadded layout gave half B's matmuls a false
    WAR against half A's MAX8 read, serializing the PE). Per pair:
    one ACT copy writes the bias into both halves (600 elems, halves
    the per-instruction overhead), 16 accumulating matmuls fill the
    halves, DVE runs MAX8 per half, Pool packs [A0..A3,B0..B3] into an
    8-wide key buffer, and ONE FIND_INDEX8 scans the 600-elem pair:
    out[0] = argmax of half A, out[4] = argmax of half B + 300.
    This keeps DVE (~1.84us/pair) under the PE's ~2.03us/pair budget -
    with per-tile FIND the DVE was the co-bottleneck.
  - Pool extracts tokens: even tiles pass through, odd tiles compute
    max(idx,300)-300 (wrap-safe: if half B's max value bit-exactly
    collides with an earlier value in half A - expected ~2 rows per
    full run - the token clamps to 0 instead of wrapping negative).
  - a dummy ACTIVATE right after the const DMA issues pulls the
    1.3us ACT_TABLE_LOAD off the first bias-copy's critical path.
  - startup is DMA-bandwidth-bound (~330GB/s shared by all rings), so
    all startup loads ride the sync ring's single in-order pipe in
    exact first-use order: ct0-1, x tile0, ct2-4, x tile1, ct5-7,
    x tiles 2-3, then the steady groups. The bias rides as a 4.8KB
    fp16 hi/lo pair [2, 600] broadcast by a 2-row PE matmul into a
    warm PSUM tile and ACT-copied once to SBUF - 300KB of startup DMA
    replaced by two 125ns matmuls.
  - group 0 is stored tile-major (xg0) and arrives as 4 single-tile
    DMAs: the first tile's full x^T lands ~256KB after the queue opens,
    so the PE reaches full rate ~5us sooner than with chunk-major
    group-0 loads (where tile 0 needed all 2MB).

Row interleaving: row-tile t holds rows {p*128 + t}, so the token buffer
[p, t] DMAs out contiguously in original row order.

Walrus only lowers one sync wait per instruction; _hoist_excess_waits
moves Tile's extra waits onto same-engine no-ops at the same program
point. Mid-kernel x loads share the sync HWDGE ring (same-ring
transfers complete in order, so prefetch can't starve urgent loads);
constants and token stores ride the scalar ring. Keep KM_HW_LANES=8:
with fewer lanes the scalar ring's startup DMAs serialize behind each
other's transfers (lane-reuse WAW), costing ~6us.
"""

import os
import sys

import numpy as np

if "/opt/trn_rl_repo" not in sys.path:
    sys.path.insert(0, "/opt/trn_rl_repo")

import concourse.bass as bass
import concourse.mybir as mybir
import concourse.tile_sem_assignment as _tsa
from concourse.bass_utils import run_bass_kernel_spmd
from concourse.tile import TileContext

_tsa.NUM_HWDGE_SEMS = int(os.environ.get("KM_HW_LANES", "8"))

# Give each HWDGE ring (SP-issued vs ACT-issued DMAs) a disjoint pool of
# completion lanes. Tile's global round-robin otherwise interleaves the
# two rings onto shared lanes, and the lane-order WAW waits then falsely
# serialize one ring behind the other.
_orig_assign_tick = _tsa.TileClockTick._assign_tick


def _assign_tick_lanepools(self, inst):
    try:
        if isinstance(inst, _tsa.DMAInst) and inst.engine != mybir.EngineType.Pool:
            if not hasattr(self, "_lane_ctr"):
                self._lane_ctr = {}
            eng = inst.engine
            n = _tsa.NUM_HWDGE_SEMS
            half = max(1, n // 2)
            pool = (
                list(range(0, half))
                if eng == mybir.EngineType.Activation
                else list(range(half, n))
            )
            c = self._lane_ctr.get(eng, 0)
            self.next_hw_dma_idx = pool[c % len(pool)]
            self._lane_ctr[eng] = c + 1
    except Exception:
        pass
    return _orig_assign_tick(self, inst)


_tsa.TileClockTick._assign_tick = _assign_tick_lanepools

P = 128
D = 1024
K = 300
NCORES = 8
ROWS = 16384            # rows per core
TILES = ROWS // P       # 128 row-tiles per core
PAIRS = TILES // 2      # 64 PSUM pair-tiles per core
HWC = 512               # fp32 cols per PSUM half (2KB = one bank)
GROUPS = 32             # DMA groups per core (1 group = 1 MB fp16)
TPG = TILES // GROUPS   # 8 row-tiles per group
DCH = D // P            # 8 contraction chunks

F16 = mybir.dt.float16
F32 = mybir.dt.float32
I32 = mybir.dt.int32
U32 = mybir.dt.uint32

# Set by kernel() so test.py can read profiling info.
LAST_RESULT = None


def _ensure_ntff_hook():
    """Install antenv.axon_hooks shim so trace=True works under axon."""
    try:
        from antenv.axon_hooks import get_axon_ntff_profile_hook  # noqa: F401

        return
    except ImportError:
        pass
    import types

    import antenv

    try:
        from trn_agent_boot.trn_boot import _ntff_profile_via_ctypes
    except ImportError:
        return
    mod = types.ModuleType("antenv.axon_hooks")
    _hook = [None]
    mod.set_axon_ntff_profile_hook = lambda h: _hook.__setitem__(0, h)
    mod.get_axon_ntff_profile_hook = lambda: _hook[0]
    sys.modules["antenv.axon_hooks"] = mod
    antenv.axon_hooks = mod
    so = "/opt/axon/libaxon_pjrt.so"
    if os.path.exists(so):
        mod.set_axon_ntff_profile_hook(_ntff_profile_via_ctypes(so))


# Token flush boundaries (exclusive tile index, all even): 16-tile blocks
# through t=112, then 8/6/2 so the final CAST+DMA is tiny and the tail
# is short.
FLUSH = [16, 32, 48, 64, 80, 96, 112, 120, 126, 128]


def _max_index_nd(nc, out, in_max, in_values):
    """max_index with a multi-dim in_values AP (bass asserts 2D, the DVE
    ISA takes general APs; indices are in AP traversal order)."""
    eng = nc.vector
    return eng.add_instruction(
        mybir.InstMaxIndex(
            name=nc.get_next_instruction_name(),
            ins=[eng.lower_ap(in_max), eng.lower_ap(in_values)],
            outs=[eng.lower_ap(out)],
        )
    )


def _emit_flush(nc, out, tokbuf, idxbuf, bounds):
    """Pool extracts tokens for tiles [s, e) and the scalar ring DMAs
    them out. Even tiles pass through; odd tiles un-offset wrap-safely:
    max(idx, 300) - 300 clamps the rare cross-half value collision to
    token 0 instead of wrapping negative."""
    s, e = bounds
    s2, e2 = s // 2, e // 2
    nc.gpsimd.tensor_copy(out=tokbuf[:, s:e:2], in_=idxbuf[:, s2:e2, 0])
    nc.gpsimd.tensor_scalar(
        tokbuf[:, s + 1 : e : 2],
        idxbuf[:, s2:e2, 4],
        300,
        300,
        mybir.AluOpType.max,
        mybir.AluOpType.subtract,
    )
    nc.scalar.dma_start(out=out[:, s:e], in_=tokbuf[:, s:e])


def _emit_last_pair_singles(nc, psp, mxp, chunk, ctiles, bft, out, tokbuf, idxbuf):
    """Tiles 126/127 run the per-tile path (own MAX8 + FIND over 300):
    tile 126's FIND overlaps tile 127's matmuls, and the post-last-matmul
    chain drops the pair-FIND's extra ~0.6us, shortening the graded tail."""
    for half in range(2):
        t = TILES - 2 + half
        ps = psp.tile([P, 2, HWC], F32, name="ps")
        nc.scalar.copy(out=ps[:, 0, :K], in_=bft[:, 0, :])
        for j in range(DCH):
            nc.tensor.matmul(
                ps[:, 0, :K],
                lhsT=chunk(j, (TPG - 2) + half),
                rhs=ctiles[j][:],
                start=False,
                stop=(j == DCH - 1),
                skip_group_check=True,
            )
        mxt = mxp.tile([P, 2, 8], F32)
        nc.vector.max(out=mxt[:, 0, :], in_=ps[:, 0, :K])
        nc.vector.max_index(
            out=idxbuf[:, PAIRS - 1 + half, :], in_max=mxt[:, 0, :],
            in_values=ps[:, 0, :K],
        )
        nc.gpsimd.tensor_copy(
            out=tokbuf[:, t : t + 1], in_=idxbuf[:, PAIRS - 1 + half, 0:1]
        )
    nc.scalar.dma_start(out=out[:, TILES - 2 :], in_=tokbuf[:, TILES - 2 :])


def build_nc() -> bass.Bass:
    n_warm = int(os.environ.get("KM_WARM", "10"))
    spread0 = bool(int(os.environ.get("KM_SPREAD0", "1")))
    table_hoist = bool(int(os.environ.get("KM_TABLE_HOIST", "1")))

    nc = bass.Bass()

    xg = nc.declare_dram_parameter("xg", [GROUPS, P, DCH * TPG * P], F16, isOutput=False)
    xg0 = nc.declare_dram_parameter("xg0", [TPG, P, DCH * P], F16, isOutput=False)
    cons = nc.declare_dram_parameter("cons", [P, DCH * K], F16, isOutput=False)
    bias2 = nc.declare_dram_parameter("bias2", [2, 2 * K], F16, isOutput=False)
    out = nc.declare_dram_parameter("out", [P, TILES], I32, isOutput=True)

    with TileContext(nc) as tc:
        with (
            tc.tile_pool(name="const", bufs=1) as constp,
            tc.tile_pool(name="warm", bufs=1) as warmp,
            tc.tile_pool(name="xp0", bufs=TPG) as xp0,
            tc.tile_pool(name="xp", bufs=5) as xp,
            tc.tile_pool(name="mx", bufs=8) as mxp,
            tc.tile_pool(name="mx8", bufs=4) as mx8p,
            tc.tile_pool(name="psum", bufs=4, space="PSUM") as psp,
            tc.tile_pool(name="outp", bufs=1) as outp,
        ):
            # everything rides the sync ring's in-order pipe in
            # first-use order, starting with the tiny bias pair
            b2t = constp.tile([2, 2 * K], F16)
            nc.sync.dma_start(out=b2t[:], in_=bias2[:])
            cons_t = constp.tile([P, DCH * K], F16)
            ctiles = [cons_t[:, j * K : (j + 1) * K] for j in range(DCH)]

            warm = warmp.tile([P, K], F16)
            nc.gpsimd.memset(warm[:], 0.0)
            ones2 = warmp.tile([2, P], F16)
            nc.gpsimd.memset(ones2[:], 1.0)
            if table_hoist:
                # first InstActivation triggers the 1.3us ACT_TABLE_LOAD;
                # issue a dummy now so it overlaps the const DMAs instead
                # of gating the first bias copy
                tdum = warmp.tile([P, 1], F32)
                nc.scalar.copy(out=tdum[:], in_=warm[:, :1])

            # PE warmup: dep-free matmuls over a memset SBUF tile into
            # the first pair tiles (overwritten by the bias broadcasts
            # below). They run during the startup DMA wait, ramping the
            # PE p-state so the first real matmuls start at full clock.
            wtiles = [psp.tile([P, 2, HWC], F32, name="ps") for _ in range(4)]
            for w in range(n_warm):
                nc.tensor.matmul(
                    wtiles[(w // 2) % 4][:, w % 2, :K],
                    lhsT=warm[:, :P], rhs=warm[:, :K],
                    start=True, stop=True,
                )
            # extra warms (w>=8) re-warm pairs 0/1 before their bias
            # broadcasts run; ordering is same-tile WAW, handled by Tile

            bft = constp.tile([P, 2, K], F32)

            # sync-ring in-order pipe: group-0 x tiles and consts in
            # exact first-use order
            xch0 = []

            def _load_x0(tl):
                cbuf = xp0.tile([P, DCH, P], F16, name="xtile")
                nc.sync.dma_start(
                    out=cbuf[:], in_=xg0[tl].rearrange("p (j q) -> p j q", j=DCH)
                )
                xch0.append(cbuf)

            _load_x0(0)
            nc.sync.dma_start(out=cons_t[:, : 2 * K], in_=cons[:, : 2 * K])
            _load_x0(1)
            nc.sync.dma_start(out=cons_t[:, 2 * K : 5 * K], in_=cons[:, 2 * K : 5 * K])
            _load_x0(2)
            nc.sync.dma_start(out=cons_t[:, 5 * K :], in_=cons[:, 5 * K :])
            _load_x0(3)

            idxbuf = outp.tile([P, PAIRS + 1, 8], U32)
            tokbuf = outp.tile([P, TILES], I32)
            pending_flush = None

            for g in range(GROUPS):
                if g == 0:
                    chunk = lambda j, tl: xch0[tl][:, j, :]
                else:
                    # all steady-state x loads share the sync ring:
                    # same-ring transfers serialize per DMA-engine FIFO,
                    # so prefetch can't steal bandwidth from earlier
                    # (more urgent) loads
                    xbuf = xp.tile([P, DCH, TPG, P], F16, name="xgrp")
                    nc.sync.dma_start(
                        out=xbuf[:],
                        in_=xg[g].rearrange("p (j t q) -> p j t q", j=DCH, t=TPG),
                    )
                    chunk = lambda j, tl, xbuf=xbuf: xbuf[:, j, tl, :]
                for pl in range(TPG // 2):
                    pr = g * (TPG // 2) + pl
                    flush_now, pending_flush = pending_flush, None
                    if pr == PAIRS - 1:
                        if flush_now is not None:
                            _emit_flush(nc, out, tokbuf, idxbuf, flush_now)
                        _emit_last_pair_singles(
                            nc, psp, mxp, chunk, ctiles, bft, out, tokbuf,
                            idxbuf,
                        )
                        continue
                    if pr < 4:
                        # pairs 0-3: bias arrives via two PE broadcast
                        # matmuls (ones2^T @ [bh; bl] accumulates bh+bl
                        # in fp32) - start=True also sets the half-bank's
                        # has_written bits, and the short PE chain beats
                        # waiting for an ACT round-trip at startup
                        ps = wtiles[pr]
                        for h in range(2):
                            nc.tensor.matmul(
                                ps[:, h, :K],
                                lhsT=ones2[:], rhs=b2t[:, h * K : (h + 1) * K],
                                start=True, stop=True,
                            )
                        if pr == 2:
                            # snapshot the bias into SBUF for pairs 4+
                            # (before this pair's matmuls clobber it)
                            nc.scalar.copy(out=bft[:], in_=ps[:, :, :K])
                    else:
                        ps = psp.tile([P, 2, HWC], F32, name="ps")
                        # has_written bits persist from this bank's prior
                        # occupant; ScalarE resets the values to the bias
                        # and the start=False matmuls accumulate on top
                        nc.scalar.copy(out=ps[:, :, :K], in_=bft[:])
                    mxt = mxp.tile([P, 2, 8], F32)
                    for half in range(2):
                        tl = 2 * pl + half
                        for j in range(DCH):
                            nc.tensor.matmul(
                                ps[:, half, :K],
                                lhsT=chunk(j, tl),
                                rhs=ctiles[j][:],
                                start=False,
                                stop=(j == DCH - 1),
                                skip_group_check=True,
                            )
                        nc.vector.max(out=mxt[:, half, :], in_=ps[:, half, :K])
                    # Pool packs the FIND keys [A0..A3, B0..B3]; only
                    # lanes 0 (maxA) and 4 (maxB) are consumed, the rest
                    # are benign real values that keep every lane written
                    mx8 = mx8p.tile([P, 8], F32)
                    nc.gpsimd.tensor_copy(out=mx8[:], in_=mxt[:, :, 0:4])
                    # one FIND over the 600-elem pair: out[0] = idx of
                    # maxA (in [0,300)), out[4] = idx of maxB + 300
                    _max_index_nd(
                        nc, out=idxbuf[:, pr, :], in_max=mx8[:],
                        in_values=ps[:, :, :K],
                    )
                    if flush_now is not None:
                        _emit_flush(nc, out, tokbuf, idxbuf, flush_now)
                    # queue this pair's flush for emission one pair later:
                    # emitting it here would park the token-DMA issue in the
                    # scalar queue ahead of the NEXT pair's bias ACTIVATE,
                    # and the in-order queue then stalls the PE on FIND
                    t = 2 * pr + 1
                    if (t + 1) in FLUSH:
                        s = FLUSH[FLUSH.index(t + 1) - 1] if (t + 1) != FLUSH[0] else 0
                        if t + 1 == TILES:
                            _emit_flush(nc, out, tokbuf, idxbuf, (s, t + 1))
                        else:
                            pending_flush = (s, t + 1)

    _hoist_excess_waits(nc)
    return nc


def _hoist_excess_waits(nc: bass.Bass, max_waits: int = 1):
    """Hoist excess sync waits onto no-op drains inserted just before.

    Walrus's codegen caps embedded sync waits per instruction (1 for
    DIRECT2D DMAs and CTRL ops), but Tile can attach several (slot-reuse
    WAR + lane WAW, or the kernel-tail drain waiting on every proc).
    A same-engine drain immediately before the instruction blocks the
    sequencer at the same program point, so semantics are unchanged.
    """
    n = 0
    for f in nc.m.functions:
        for blk in f.blocks:
            insts = blk.instructions
            i = 0
            while i < len(insts):
                inst = insts[i]
                si = inst.sync_info
                if si and si.on_wait and len(si.on_wait) > max_waits:
                    waits = list(si.on_wait)
                    si.on_wait = waits[-max_waits:]
                    inst.sync_info = si
                    pre = []
                    for j in range(0, len(waits) - max_waits, max_waits):
                        nd = mybir.InstNoOp(name=f"I-wsplit{n}", ins=[], outs=[])
                        n += 1
                        nd.engine = inst.engine
                        nsi = type(si)(
                            on_wait=waits[j : j + max_waits], on_update=[]
                        )
                        nd.sync_info = nsi
                        try:
                            nc.register_instruction(nd, overwrite=True)
                        except Exception:
                            pass
                        pre.append(nd)
                    for k, nd in enumerate(pre):
                        insts.insert(i + k, nd)
                    i += len(pre)
                i += 1


def make_in_maps(x: np.ndarray, C: np.ndarray, Cnorm: np.ndarray):
    x16 = x.astype(np.float16)
    C16 = C.astype(np.float16).reshape(DCH, P, K)

    cons = np.ascontiguousarray(C16.transpose(1, 0, 2).reshape(P, DCH * K))
    b1 = (-0.5 * Cnorm.reshape(K)).astype(np.float32)
    bh = b1.astype(np.float16)
    bl = (b1 - bh.astype(np.float32)).astype(np.float16)
    bias2 = np.stack([np.concatenate([bh, bh]), np.concatenate([bl, bl])])

    in_maps = []
    for c in range(NCORES):
        xs = x16[c * ROWS : (c + 1) * ROWS]
        # row r = p*128 + g*TPG + tl ; col = j*128 + pd
        xr = xs.reshape(P, GROUPS, TPG, DCH, P)          # [p, g, tl, j, pd]
        xgc = np.ascontiguousarray(xr.transpose(1, 4, 3, 2, 0))  # [g, pd, j, tl, p]
        xg0 = np.ascontiguousarray(xr[:, 0].transpose(1, 3, 2, 0))  # [tl, pd, j, p]
        in_maps.append(
            {
                "xg": xgc.reshape(GROUPS, P, DCH * TPG * P),
                "xg0": xg0.reshape(TPG, P, DCH * P),
                "cons": cons,
                "bias2": bias2,
            }
        )
    return in_maps


_NC_CACHE = {}


def kernel(x, C, Cnorm, b, t):
    global LAST_RESULT
    x = np.asarray(x)
    C = np.asarray(C)
    Cnorm = np.asarray(Cnorm)

    key = 0
    if key not in _NC_CACHE:
        _NC_CACHE[key] = build_nc()
    nc = _NC_CACHE[key]

    in_maps = make_in_maps(x, C, Cnorm)
    trace = bool(int(os.environ.get("KM_TRACE", "0")))
    if trace:
        _ensure_ntff_hook()
    res = run_bass_kernel_spmd(
        nc, in_maps, core_ids=list(range(NCORES)), trace=trace
    )
    LAST_RESULT = res

    shards = [res.results[c]["out"].reshape(-1) for c in range(NCORES)]
    tokens = np.concatenate(shards).astype(np.int32)
    return tokens.reshape(int(b), int(t))


# revision 20
# speedup vs baseline: 1.0107x; 1.0014x over previous
"""VQ codebook assignment (ApplyKmeans) on 8 Trainium2 NeuronCores.

tokens[n] = argmin_k ||x_n - c_k||^2
          = argmax_k (x_n.c_k - Cnorm_k/2)        (||x_n||^2 constant per row)

Data-parallel: x sharded along N across 8 cores, C/Cnorm replicated.

Per core (16384 rows, 128 row-tiles of 128 rows):
  - host pre-tiles x^T so each [128d, 128n] stationary tile is contiguous
    (fp16: halves HBM traffic; PSUM accumulates fp32; ~52/131072 argmin
    flips vs the fp32 reference, rel err ~0.0144)
  - warmup: dep-free matmuls over a memset SBUF tile, cycling the PSUM
    pair-tiles. They execute during the initial DMA wait, ramping the
    PE out of its low p-state, and their start=True writes set every
    PSUM has_written bit - so every real tile uses the ACT-copy bias
    path (ScalarE rewrites the bank to -Cnorm/2, then 8 start=False
    matmuls accumulate on top). No bias matmuls needed.
  - row-tiles are processed in PAIRS sharing one 2-bank PSUM tile
    [128, 2, 512], each half bank-aligned (Tile's dependency tracking
    is bank-granular: an unpadded layout gave half B's matmuls a false
    WAR against half A's MAX8 read, serializing the PE). Per pair:
    one ACT copy writes the bias into both halves (600 elems, halves
    the per-instruction overhead), 16 accumulating matmuls fill the
    halves, DVE runs MAX8 per half, Pool packs [A0..A3,B0..B3] into an
    8-wide key buffer, and ONE FIND_INDEX8 scans the 600-elem pair:
    out[0] = argmax of half A, out[4] = argmax of half B + 300.
    This keeps DVE (~1.84us/pair) under the PE's ~2.03us/pair budget -
    with per-tile FIND the DVE was the co-bottleneck.
  - Pool extracts tokens: even tiles pass through, odd tiles compute
    max(idx,300)-300 (wrap-safe: if half B's max value bit-exactly
    collides with an earlier value in half A - expected ~2 rows per
    full run - the token clamps to 0 instead of wrapping negative).
  - a dummy ACTIVATE right after the const DMA issues pulls the
    1.3us ACT_TABLE_LOAD off the first bias-copy's critical path.
  - startup is DMA-bandwidth-bound (~330GB/s shared by all rings), so
    all startup loads ride the sync ring's single in-order pipe in
    exact first-use order: ct0-1, x tile0, ct2-4, x tile1, ct5-7,
    x tiles 2-3, then the steady groups. The bias rides as a 4.8KB
    fp16 hi/lo pair [2, 600] broadcast by a 2-row PE matmul into a
    warm PSUM tile and ACT-copied once to SBUF - 300KB of startup DMA
    replaced by two 125ns matmuls.
  - group 0 is stored tile-major (xg0) and arrives as 4 single-tile
    DMAs: the first tile's full x^T lands ~256KB after the queue opens,
    so the PE reaches full rate ~5us sooner than with chunk-major
    group-0 loads (where tile 0 needed all 2MB).

Row interleaving: row-tile t holds rows {p*128 + t}, so the token buffer
[p, t] DMAs out contiguously in original row order.

Walrus only lowers one sync wait per instruction; _hoist_excess_waits
moves Tile's extra waits onto same-engine no-ops at the same program
point. Mid-kernel x loads share the sync HWDGE ring (same-ring
transfers complete in order, so prefetch can't starve urgent loads);
constants and token stores ride the scalar ring. Keep KM_HW_LANES=8:
with fewer lanes the scalar ring's startup DMAs serialize behind each
other's transfers (lane-reuse WAW), costing ~6us.
"""

import os
import sys

import numpy as np

if "/opt/trn_rl_repo" not in sys.path:
    sys.path.insert(0, "/opt/trn_rl_repo")

import concourse.bass as bass
import concourse.mybir as mybir
import concourse.tile_sem_assignment as _tsa
from concourse.bass_utils import run_bass_kernel_spmd
from concourse.tile import TileContext

_tsa.NUM_HWDGE_SEMS = int(os.environ.get("KM_HW_LANES", "8"))

# Give each HWDGE ring (SP-issued vs ACT-issued DMAs) a disjoint pool of
# completion lanes. Tile's global round-robin otherwise interleaves the
# two rings onto shared lanes, and the lane-order WAW waits then falsely
# serialize one ring behind the other.
_orig_assign_tick = _tsa.TileClockTick._assign_tick


def _assign_tick_lanepools(self, inst):
    try:
        if isinstance(inst, _tsa.DMAInst) and inst.engine != mybir.EngineType.Pool:
            if not hasattr(self, "_lane_ctr"):
                self._lane_ctr = {}
            eng = inst.engine
            n = _tsa.NUM_HWDGE_SEMS
            half = max(1, n // 2)
            pool = (
                list(range(0, half))
                if eng == mybir.EngineType.Activation
                else list(range(half, n))
            )
            c = self._lane_ctr.get(eng, 0)
            self.next_hw_dma_idx = pool[c % len(pool)]
            self._lane_ctr[eng] = c + 1
    except Exception:
        pass
    return _orig_assign_tick(self, inst)


_tsa.TileClockTick._assign_tick = _assign_tick_lanepools

P = 128
D = 1024
K = 300
NCORES = 8
ROWS = 16384            # rows per core
TILES = ROWS // P       # 128 row-tiles per core
PAIRS = TILES // 2      # 64 PSUM pair-tiles per core
HWC = 512               # fp32 cols per PSUM half (2KB = one bank)
GROUPS = 32             # DMA groups per core (1 group = 1 MB fp16)
TPG = TILES // GROUPS   # 8 row-tiles per group
DCH = D // P            # 8 contraction chunks

F16 = mybir.dt.float16
F32 = mybir.dt.float32
I32 = mybir.dt.int32
U32 = mybir.dt.uint32

# Set by kernel() so test.py can read profiling info.
LAST_RESULT = None


def _ensure_ntff_hook():
    """Install antenv.axon_hooks shim so trace=True works under axon."""
    try:
        from antenv.axon_hooks import get_axon_ntff_profile_hook  # noqa: F401

        return
    except ImportError:
        pass
    import types

    import antenv

    try:
        from trn_agent_boot.trn_boot import _ntff_profile_via_ctypes
    except ImportError:
        return
    mod = types.ModuleType("antenv.axon_hooks")
    _hook = [None]
    mod.set_axon_ntff_profile_hook = lambda h: _hook.__setitem__(0, h)
    mod.get_axon_ntff_profile_hook = lambda: _hook[0]
    sys.modules["antenv.axon_hooks"] = mod
    antenv.axon_hooks = mod
    so = "/opt/axon/libaxon_pjrt.so"
    if os.path.exists(so):
        mod.set_axon_ntff_profile_hook(_ntff_profile_via_ctypes(so))


# Token flush boundaries (exclusive tile index, all even): 16-tile blocks
# through t=112, then 8/6/2 so the final CAST+DMA is tiny and the tail
# is short.
FLUSH = [16, 32, 48, 64, 80, 96, 112, 120, 126, 128]


def _max_index_nd(nc, out, in_max, in_values):
    """max_index with a multi-dim in_values AP (bass asserts 2D, the DVE
    ISA takes general APs; indices are in AP traversal order)."""
    eng = nc.vector
    return eng.add_instruction(
        mybir.InstMaxIndex(
            name=nc.get_next_instruction_name(),
            ins=[eng.lower_ap(in_max), eng.lower_ap(in_values)],
            outs=[eng.lower_ap(out)],
        )
    )


def _emit_flush(nc, out, tokbuf, idxbuf, bounds):
    """Pool extracts tokens for tiles [s, e) and the scalar ring DMAs
    them out. Even tiles pass through; odd tiles un-offset wrap-safely:
    max(idx, 300) - 300 clamps the rare cross-half value collision to
    token 0 instead of wrapping negative."""
    s, e = bounds
    s2, e2 = s // 2, e // 2
    nc.gpsimd.tensor_copy(out=tokbuf[:, s:e:2], in_=idxbuf[:, s2:e2, 0])
    nc.gpsimd.tensor_scalar(
        tokbuf[:, s + 1 : e : 2],
        idxbuf[:, s2:e2, 4],
        300,
        300,
        mybir.AluOpType.max,
        mybir.AluOpType.subtract,
    )
    nc.scalar.dma_start(out=out[:, s:e], in_=tokbuf[:, s:e])


def _emit_last_pair_singles(nc, psp, mxp, chunk, ctiles, bft, out, tokbuf, idxbuf):
    """Tiles 126/127 run the per-tile path (own MAX8 + FIND over 300):
    tile 126's FIND overlaps tile 127's matmuls, and the post-last-matmul
    chain drops the pair-FIND's extra ~0.6us, shortening the graded tail."""
    for half in range(2):
        t = TILES - 2 + half
        ps = psp.tile([P, 2, HWC], F32, name="ps")
        nc.scalar.copy(out=ps[:, 0, :K], in_=bft[:, 0, :])
        for j in range(DCH):
            nc.tensor.matmul(
                ps[:, 0, :K],
                lhsT=chunk(j, (TPG - 2) + half),
                rhs=ctiles[j][:],
                start=False,
                stop=(j == DCH - 1),
                skip_group_check=True,
            )
        mxt = mxp.tile([P, 2, 8], F32)
        nc.vector.max(out=mxt[:, 0, :], in_=ps[:, 0, :K])
        nc.vector.max_index(
            out=idxbuf[:, PAIRS - 1 + half, :], in_max=mxt[:, 0, :],
            in_values=ps[:, 0, :K],
        )
        nc.gpsimd.tensor_copy(
            out=tokbuf[:, t : t + 1], in_=idxbuf[:, PAIRS - 1 + half, 0:1]
        )
    nc.scalar.dma_start(out=out[:, TILES - 2 :], in_=tokbuf[:, TILES - 2 :])


def build_nc() -> bass.Bass:
    n_warm = int(os.environ.get("KM_WARM", "10"))
    spread0 = bool(int(os.environ.get("KM_SPREAD0", "1")))
    table_hoist = bool(int(os.environ.get("KM_TABLE_HOIST", "1")))

    nc = bass.Bass()

    # x arrives as: 4 tile-major singles (tiles 0-3), two 2-tile groups
    # (tiles 4-5, 6-7) bridging the supply-deficit window while the PE
    # clock ramps, then 30 4-tile groups
    xg0 = nc.declare_dram_parameter("xg0", [TPG, P, DCH * P], F16, isOutput=False)
    xgA = nc.declare_dram_parameter("xgA", [2, P, DCH * 2 * P], F16, isOutput=False)
    xg = nc.declare_dram_parameter("xg", [GROUPS - 2, P, DCH * TPG * P], F16, isOutput=False)
    cons = nc.declare_dram_parameter("cons", [P, DCH * K], F16, isOutput=False)
    bias2 = nc.declare_dram_parameter("bias2", [2, 2 * K], F16, isOutput=False)
    out = nc.declare_dram_parameter("out", [P, TILES], I32, isOutput=True)

    with TileContext(nc) as tc:
        with (
            tc.tile_pool(name="const", bufs=1) as constp,
            tc.tile_pool(name="warm", bufs=1) as warmp,
            tc.tile_pool(name="xp0", bufs=TPG) as xp0,
            tc.tile_pool(name="xp", bufs=5) as xp,
            tc.tile_pool(name="mx", bufs=8) as mxp,
            tc.tile_pool(name="mx8", bufs=4) as mx8p,
            tc.tile_pool(name="psum", bufs=4, space="PSUM") as psp,
            tc.tile_pool(name="outp", bufs=1) as outp,
        ):
            # everything rides the sync ring's in-order pipe in
            # first-use order, starting with the tiny bias pair
            b2t = constp.tile([2, 2 * K], F16)
            nc.sync.dma_start(out=b2t[:], in_=bias2[:])
            cons_t = constp.tile([P, DCH * K], F16)
            ctiles = [cons_t[:, j * K : (j + 1) * K] for j in range(DCH)]

            warm = warmp.tile([P, K], F16)
            nc.gpsimd.memset(warm[:], 0.0)
            ones2 = warmp.tile([2, P], F16)
            nc.gpsimd.memset(ones2[:], 1.0)
            if table_hoist:
                # first InstActivation triggers the 1.3us ACT_TABLE_LOAD;
                # issue a dummy now so it overlaps the const DMAs instead
                # of gating the first bias copy
                tdum = warmp.tile([P, 1], F32)
                nc.scalar.copy(out=tdum[:], in_=warm[:, :1])

            # PE warmup: dep-free matmuls over a memset SBUF tile into
            # the first pair tiles (overwritten by the bias broadcasts
            # below). They run during the startup DMA wait, ramping the
            # PE p-state so the first real matmuls start at full clock.
            wtiles = [psp.tile([P, 2, HWC], F32, name="ps") for _ in range(4)]
            for w in range(n_warm):
                nc.tensor.matmul(
                    wtiles[(w // 2) % 4][:, w % 2, :K],
                    lhsT=warm[:, :P], rhs=warm[:, :K],
                    start=True, stop=True,
                )
            # extra warms (w>=8) re-warm pairs 0/1 before their bias
            # broadcasts run; ordering is same-tile WAW, handled by Tile

            bft = constp.tile([P, 2, K], F32)

            # sync-ring in-order pipe: group-0 x tiles and consts in
            # exact first-use order
            xch0 = []

            def _load_x0(tl):
                cbuf = xp0.tile([P, DCH, P], F16, name="xtile")
                nc.sync.dma_start(
                    out=cbuf[:], in_=xg0[tl].rearrange("p (j q) -> p j q", j=DCH)
                )
                xch0.append(cbuf)

            _load_x0(0)
            nc.sync.dma_start(out=cons_t[:, : 2 * K], in_=cons[:, : 2 * K])
            _load_x0(1)
            nc.sync.dma_start(out=cons_t[:, 2 * K : 5 * K], in_=cons[:, 2 * K : 5 * K])
            _load_x0(2)
            nc.sync.dma_start(out=cons_t[:, 5 * K :], in_=cons[:, 5 * K :])
            _load_x0(3)

            idxbuf = outp.tile([P, PAIRS + 1, 8], U32)
            tokbuf = outp.tile([P, TILES], I32)
            pending_flush = None

            # schedule: (first_tile, ntiles, src) per group
            sched = [(0, TPG, None), (TPG, 2, xgA[0]), (TPG + 2, 2, xgA[1])]
            for gg in range(GROUPS - 2):
                sched.append((2 * TPG + gg * TPG, TPG, xg[gg]))
            pr = 0
            for t0g, ntl, src in sched:
                if src is None:
                    chunk = lambda j, tl: xch0[tl][:, j, :]
                else:
                    # all steady-state x loads share the sync ring:
                    # same-ring transfers serialize per DMA-engine FIFO,
                    # so prefetch can't steal bandwidth from earlier
                    # (more urgent) loads
                    xbuf = xp.tile([P, DCH, ntl, P], F16, name=f"xgrp{ntl}")
                    nc.sync.dma_start(
                        out=xbuf[:],
                        in_=src.rearrange("p (j t q) -> p j t q", j=DCH, t=ntl),
                    )
                    chunk = lambda j, tl, xbuf=xbuf: xbuf[:, j, tl, :]
                for pl in range(ntl // 2):
                    flush_now, pending_flush = pending_flush, None
                    if pr == PAIRS - 1:
                        if flush_now is not None:
                            _emit_flush(nc, out, tokbuf, idxbuf, flush_now)
                        _emit_last_pair_singles(
                            nc, psp, mxp, chunk, ctiles, bft, out, tokbuf,
                            idxbuf,
                        )
                        continue
                    if pr < 4:
                        # pairs 0-3: bias arrives via two PE broadcast
                        # matmuls (ones2^T @ [bh; bl] accumulates bh+bl
                        # in fp32) - start=True also sets the half-bank's
                        # has_written bits, and the short PE chain beats
                        # waiting for an ACT round-trip at startup
                        ps = wtiles[pr]
                        for h in range(2):
                            nc.tensor.matmul(
                                ps[:, h, :K],
                                lhsT=ones2[:], rhs=b2t[:, h * K : (h + 1) * K],
                                start=True, stop=True,
                            )
                        if pr == 2:
                            # snapshot the bias into SBUF for pairs 4+
                            # (before this pair's matmuls clobber it)
                            nc.scalar.copy(out=bft[:], in_=ps[:, :, :K])
                    else:
                        ps = psp.tile([P, 2, HWC], F32, name="ps")
                        # has_written bits persist from this bank's prior
                        # occupant; ScalarE resets the values to the bias
                        # and the start=False matmuls accumulate on top
                        nc.scalar.copy(out=ps[:, :, :K], in_=bft[:])
                    mxt = mxp.tile([P, 2, 8], F32)
                    for half in range(2):
                        tl = 2 * pl + half
                        assert t0g + tl == 2 * pr + half
                        for j in range(DCH):
                            nc.tensor.matmul(
                                ps[:, half, :K],
                                lhsT=chunk(j, tl),
                                rhs=ctiles[j][:],
                                start=False,
                                stop=(j == DCH - 1),
                                skip_group_check=True,
                            )
                        nc.vector.max(out=mxt[:, half, :], in_=ps[:, half, :K])
                    # Pool packs the FIND keys [A0..A3, B0..B3]; only
                    # lanes 0 (maxA) and 4 (maxB) are consumed, the rest
                    # are benign real values that keep every lane written
                    mx8 = mx8p.tile([P, 8], F32)
                    nc.gpsimd.tensor_copy(out=mx8[:], in_=mxt[:, :, 0:4])
                    # one FIND over the 600-elem pair: out[0] = idx of
                    # maxA (in [0,300)), out[4] = idx of maxB + 300
                    _max_index_nd(
                        nc, out=idxbuf[:, pr, :], in_max=mx8[:],
                        in_values=ps[:, :, :K],
                    )
                    if flush_now is not None:
                        _emit_flush(nc, out, tokbuf, idxbuf, flush_now)
                    # queue this pair's flush for emission one pair later:
                    # emitting it here would park the token-DMA issue in the
                    # scalar queue ahead of the NEXT pair's bias ACTIVATE,
                    # and the in-order queue then stalls the PE on FIND
                    t = 2 * pr + 1
                    if (t + 1) in FLUSH:
                        s = FLUSH[FLUSH.index(t + 1) - 1] if (t + 1) != FLUSH[0] else 0
                        if t + 1 == TILES:
                            _emit_flush(nc, out, tokbuf, idxbuf, (s, t + 1))
                        else:
                            pending_flush = (s, t + 1)
                    pr += 1

    _hoist_excess_waits(nc)
    return nc


def _hoist_excess_waits(nc: bass.Bass, max_waits: int = 1):
    """Hoist excess sync waits onto no-op drains inserted just before.

    Walrus's codegen caps embedded sync waits per instruction (1 for
    DIRECT2D DMAs and CTRL ops), but Tile can attach several (slot-reuse
    WAR + lane WAW, or the kernel-tail drain waiting on every proc).
    A same-engine drain immediately before the instruction blocks the
    sequencer at the same program point, so semantics are unchanged.
    """
    n = 0
    for f in nc.m.functions:
        for blk in f.blocks:
            insts = blk.instructions
            i = 0
            while i < len(insts):
                inst = insts[i]
                si = inst.sync_info
                if si and si.on_wait and len(si.on_wait) > max_waits:
                    waits = list(si.on_wait)
                    si.on_wait = waits[-max_waits:]
                    inst.sync_info = si
                    pre = []
                    for j in range(0, len(waits) - max_waits, max_waits):
                        nd = mybir.InstNoOp(name=f"I-wsplit{n}", ins=[], outs=[])
                        n += 1
                        nd.engine = inst.engine
                        nsi = type(si)(
                            on_wait=waits[j : j + max_waits], on_update=[]
                        )
                        nd.sync_info = nsi
                        try:
                            nc.register_instruction(nd, overwrite=True)
                        except Exception:
                            pass
                        pre.append(nd)
                    for k, nd in enumerate(pre):
                        insts.insert(i + k, nd)
                    i += len(pre)
                i += 1


def make_in_maps(x: np.ndarray, C: np.ndarray, Cnorm: np.ndarray):
    x16 = x.astype(np.float16)
    C16 = C.astype(np.float16).reshape(DCH, P, K)

    cons = np.ascontiguousarray(C16.transpose(1, 0, 2).reshape(P, DCH * K))
    b1 = (-0.5 * Cnorm.reshape(K)).astype(np.float32)
    bh = b1.astype(np.float16)
    bl = (b1 - bh.astype(np.float32)).astype(np.float16)
    bias2 = np.stack([np.concatenate([bh, bh]), np.concatenate([bl, bl])])

    in_maps = []
    for c in range(NCORES):
        xs = x16[c * ROWS : (c + 1) * ROWS]
        # row r = p*128 + g*TPG + tl ; col = j*128 + pd
        xr = xs.reshape(P, GROUPS, TPG, DCH, P)          # [p, g, tl, j, pd]
        xgc = np.ascontiguousarray(xr[:, 2:].transpose(1, 4, 3, 2, 0))  # [g, pd, j, tl, p]
        xg0 = np.ascontiguousarray(xr[:, 0].transpose(1, 3, 2, 0))  # [tl, pd, j, p]
        xgA = np.ascontiguousarray(
            xr[:, 1].reshape(P, 2, 2, DCH, P).transpose(1, 4, 3, 2, 0)
        )  # [half, pd, j, tl2, p]
        in_maps.append(
            {
                "xg": xgc.reshape(GROUPS - 2, P, DCH * TPG * P),
                "xg0": xg0.reshape(TPG, P, DCH * P),
                "xgA": xgA.reshape(2, P, DCH * 2 * P),
                "cons": cons,
                "bias2": bias2,
            }
        )
    return in_maps


_NC_CACHE = {}


def kernel(x, C, Cnorm, b, t):
    global LAST_RESULT
    x = np.asarray(x)
    C = np.asarray(C)
    Cnorm = np.asarray(Cnorm)

    key = 0
    if key not in _NC_CACHE:
        _NC_CACHE[key] = build_nc()
    nc = _NC_CACHE[key]

    in_maps = make_in_maps(x, C, Cnorm)
    trace = bool(int(os.environ.get("KM_TRACE", "0")))
    if trace:
        _ensure_ntff_hook()
    res = run_bass_kernel_spmd(
        nc, in_maps, core_ids=list(range(NCORES)), trace=trace
    )
    LAST_RESULT = res

    shards = [res.results[c]["out"].reshape(-1) for c in range(NCORES)]
    tokens = np.concatenate(shards).astype(np.int32)
    return tokens.reshape(int(b), int(t))


# revision 21
# speedup vs baseline: 1.0125x; 1.0018x over previous
"""VQ codebook assignment (ApplyKmeans) on 8 Trainium2 NeuronCores.

tokens[n] = argmin_k ||x_n - c_k||^2
          = argmax_k (x_n.c_k - Cnorm_k/2)        (||x_n||^2 constant per row)

Data-parallel: x sharded along N across 8 cores, C/Cnorm replicated.

Per core (16384 rows, 128 row-tiles of 128 rows):
  - host pre-tiles x^T so each [128d, 128n] stationary tile is contiguous
    (fp16: halves HBM traffic; PSUM accumulates fp32; ~52/131072 argmin
    flips vs the fp32 reference, rel err ~0.0144)
  - warmup: dep-free matmuls over a memset SBUF tile, cycling the PSUM
    pair-tiles. They execute during the initial DMA wait, ramping the
    PE out of its low p-state, and their start=True writes set every
    PSUM has_written bit - so every real tile uses the ACT-copy bias
    path (ScalarE rewrites the bank to -Cnorm/2, then 8 start=False
    matmuls accumulate on top). No bias matmuls needed.
  - row-tiles are processed in PAIRS sharing one 2-bank PSUM tile
    [128, 2, 512], each half bank-aligned (Tile's dependency tracking
    is bank-granular: an unpadded layout gave half B's matmuls a false
    WAR against half A's MAX8 read, serializing the PE). Per pair:
    one ACT copy writes the bias into both halves (600 elems, halves
    the per-instruction overhead), 16 accumulating matmuls fill the
    halves, DVE runs MAX8 per half, Pool packs [A0..A3,B0..B3] into an
    8-wide key buffer, and ONE FIND_INDEX8 scans the 600-elem pair:
    out[0] = argmax of half A, out[4] = argmax of half B + 300.
    This keeps DVE (~1.84us/pair) under the PE's ~2.03us/pair budget -
    with per-tile FIND the DVE was the co-bottleneck.
  - Pool extracts tokens: even tiles pass through, odd tiles compute
    max(idx,300)-300 (wrap-safe: if half B's max value bit-exactly
    collides with an earlier value in half A - expected ~2 rows per
    full run - the token clamps to 0 instead of wrapping negative).
  - a dummy ACTIVATE right after the const DMA issues pulls the
    1.3us ACT_TABLE_LOAD off the first bias-copy's critical path.
  - startup is DMA-bandwidth-bound (~330GB/s shared by all rings), so
    all startup loads ride the sync ring's single in-order pipe in
    exact first-use order: ct0-1, x tile0, ct2-4, x tile1, ct5-7,
    x tiles 2-3, then the steady groups. The bias rides as a 4.8KB
    fp16 hi/lo pair [2, 600] broadcast by a 2-row PE matmul into a
    warm PSUM tile and ACT-copied once to SBUF - 300KB of startup DMA
    replaced by two 125ns matmuls.
  - group 0 is stored tile-major (xg0) and arrives as 4 single-tile
    DMAs: the first tile's full x^T lands ~256KB after the queue opens,
    so the PE reaches full rate ~5us sooner than with chunk-major
    group-0 loads (where tile 0 needed all 2MB).

Row interleaving: row-tile t holds rows {p*128 + t}, so the token buffer
[p, t] DMAs out contiguously in original row order.

Walrus only lowers one sync wait per instruction; _hoist_excess_waits
moves Tile's extra waits onto same-engine no-ops at the same program
point. Mid-kernel x loads share the sync HWDGE ring (same-ring
transfers complete in order, so prefetch can't starve urgent loads);
constants and token stores ride the scalar ring. Keep KM_HW_LANES=8:
with fewer lanes the scalar ring's startup DMAs serialize behind each
other's transfers (lane-reuse WAW), costing ~6us.
"""

import os
import sys

import numpy as np

if "/opt/trn_rl_repo" not in sys.path:
    sys.path.insert(0, "/opt/trn_rl_repo")

import concourse.bass as bass
import concourse.mybir as mybir
import concourse.tile_sem_assignment as _tsa
from concourse.bass_utils import run_bass_kernel_spmd
from concourse.tile import TileContext

_tsa.NUM_HWDGE_SEMS = int(os.environ.get("KM_HW_LANES", "8"))

# Give each HWDGE ring (SP-issued vs ACT-issued DMAs) a disjoint pool of
# completion lanes. Tile's global round-robin otherwise interleaves the
# two rings onto shared lanes, and the lane-order WAW waits then falsely
# serialize one ring behind the other.
_orig_assign_tick = _tsa.TileClockTick._assign_tick


def _assign_tick_lanepools(self, inst):
    try:
        if isinstance(inst, _tsa.DMAInst) and inst.engine != mybir.EngineType.Pool:
            if not hasattr(self, "_lane_ctr"):
                self._lane_ctr = {}
            eng = inst.engine
            n = _tsa.NUM_HWDGE_SEMS
            half = max(1, n // 2)
            pool = (
                list(range(0, half))
                if eng == mybir.EngineType.Activation
                else list(range(half, n))
            )
            c = self._lane_ctr.get(eng, 0)
            self.next_hw_dma_idx = pool[c % len(pool)]
            self._lane_ctr[eng] = c + 1
    except Exception:
        pass
    return _orig_assign_tick(self, inst)


_tsa.TileClockTick._assign_tick = _assign_tick_lanepools

P = 128
D = 1024
K = 300
NCORES = 8
ROWS = 16384            # rows per core
TILES = ROWS // P       # 128 row-tiles per core
PAIRS = TILES // 2      # 64 PSUM pair-tiles per core
HWC = 512               # fp32 cols per PSUM half (2KB = one bank)
GROUPS = 32             # DMA groups per core (1 group = 1 MB fp16)
TPG = TILES // GROUPS   # 8 row-tiles per group
DCH = D // P            # 8 contraction chunks

F16 = mybir.dt.float16
F32 = mybir.dt.float32
I32 = mybir.dt.int32
U32 = mybir.dt.uint32

# Set by kernel() so test.py can read profiling info.
LAST_RESULT = None


def _ensure_ntff_hook():
    """Install antenv.axon_hooks shim so trace=True works under axon."""
    try:
        from antenv.axon_hooks import get_axon_ntff_profile_hook  # noqa: F401

        return
    except ImportError:
        pass
    import types

    import antenv

    try:
        from trn_agent_boot.trn_boot import _ntff_profile_via_ctypes
    except ImportError:
        return
    mod = types.ModuleType("antenv.axon_hooks")
    _hook = [None]
    mod.set_axon_ntff_profile_hook = lambda h: _hook.__setitem__(0, h)
    mod.get_axon_ntff_profile_hook = lambda: _hook[0]
    sys.modules["antenv.axon_hooks"] = mod
    antenv.axon_hooks = mod
    so = "/opt/axon/libaxon_pjrt.so"
    if os.path.exists(so):
        mod.set_axon_ntff_profile_hook(_ntff_profile_via_ctypes(so))


# Token flush boundaries (exclusive tile index, all even): 16-tile blocks
# through t=112, then 8/6/2 so the final CAST+DMA is tiny and the tail
# is short.
FLUSH = [16, 32, 48, 64, 80, 96, 112, 120, 126, 128]


def _max_index_nd(nc, out, in_max, in_values):
    """max_index with a multi-dim in_values AP (bass asserts 2D, the DVE
    ISA takes general APs; indices are in AP traversal order)."""
    eng = nc.vector
    return eng.add_instruction(
        mybir.InstMaxIndex(
            name=nc.get_next_instruction_name(),
            ins=[eng.lower_ap(in_max), eng.lower_ap(in_values)],
            outs=[eng.lower_ap(out)],
        )
    )


def _emit_flush(nc, out, tokbuf, idxbuf, bounds):
    """Pool extracts tokens for tiles [s, e) and the scalar ring DMAs
    them out. Even tiles pass through; odd tiles un-offset wrap-safely:
    max(idx, 300) - 300 clamps the rare cross-half value collision to
    token 0 instead of wrapping negative."""
    s, e = bounds
    s2, e2 = s // 2, e // 2
    nc.gpsimd.tensor_copy(out=tokbuf[:, s:e:2], in_=idxbuf[:, s2:e2, 0])
    nc.gpsimd.tensor_scalar(
        tokbuf[:, s + 1 : e : 2],
        idxbuf[:, s2:e2, 4],
        300,
        300,
        mybir.AluOpType.max,
        mybir.AluOpType.subtract,
    )
    nc.scalar.dma_start(out=out[:, s:e], in_=tokbuf[:, s:e])


def _emit_last_pair_singles(nc, psp, mxp, chunk, ctiles, bft, out, tokbuf, idxbuf):
    """Tiles 126/127 run the per-tile path (own MAX8 + FIND over 300):
    tile 126's FIND overlaps tile 127's matmuls, and the post-last-matmul
    chain drops the pair-FIND's extra ~0.6us, shortening the graded tail."""
    for half in range(2):
        t = TILES - 2 + half
        ps = psp.tile([P, 2, HWC], F32, name="ps")
        nc.scalar.copy(out=ps[:, 0, :K], in_=bft[:, 0, :])
        for j in range(DCH):
            nc.tensor.matmul(
                ps[:, 0, :K],
                lhsT=chunk(j, (TPG - 2) + half),
                rhs=ctiles[j][:],
                start=False,
                stop=(j == DCH - 1),
                skip_group_check=True,
            )
        mxt = mxp.tile([P, 2, 8], F32)
        nc.vector.max(out=mxt[:, 0, :], in_=ps[:, 0, :K])
        nc.vector.max_index(
            out=idxbuf[:, PAIRS - 1 + half, :], in_max=mxt[:, 0, :],
            in_values=ps[:, 0, :K],
        )
        nc.gpsimd.tensor_copy(
            out=tokbuf[:, t : t + 1], in_=idxbuf[:, PAIRS - 1 + half, 0:1]
        )
    nc.scalar.dma_start(out=out[:, TILES - 2 :], in_=tokbuf[:, TILES - 2 :])


def build_nc() -> bass.Bass:
    n_warm = int(os.environ.get("KM_WARM", "14"))
    spread0 = bool(int(os.environ.get("KM_SPREAD0", "1")))
    table_hoist = bool(int(os.environ.get("KM_TABLE_HOIST", "1")))

    nc = bass.Bass()

    # x arrives as: 4 tile-major singles (tiles 0-3), two 2-tile groups
    # (tiles 4-5, 6-7) bridging the supply-deficit window while the PE
    # clock ramps, then 30 4-tile groups
    xg0 = nc.declare_dram_parameter("xg0", [TPG, P, DCH * P], F16, isOutput=False)
    xgA = nc.declare_dram_parameter("xgA", [2, P, DCH * 2 * P], F16, isOutput=False)
    xg = nc.declare_dram_parameter("xg", [GROUPS - 2, P, DCH * TPG * P], F16, isOutput=False)
    cons = nc.declare_dram_parameter("cons", [P, DCH * K], F16, isOutput=False)
    bias2 = nc.declare_dram_parameter("bias2", [2, 2 * K], F16, isOutput=False)
    out = nc.declare_dram_parameter("out", [P, TILES], I32, isOutput=True)

    with TileContext(nc) as tc:
        with (
            tc.tile_pool(name="const", bufs=1) as constp,
            tc.tile_pool(name="warm", bufs=1) as warmp,
            tc.tile_pool(name="xp0", bufs=TPG) as xp0,
            tc.tile_pool(name="xp", bufs=5) as xp,
            tc.tile_pool(name="mx", bufs=8) as mxp,
            tc.tile_pool(name="mx8", bufs=4) as mx8p,
            tc.tile_pool(name="psum", bufs=4, space="PSUM") as psp,
            tc.tile_pool(name="outp", bufs=1) as outp,
        ):
            # everything rides the sync ring's in-order pipe in
            # first-use order, starting with the tiny bias pair
            b2t = constp.tile([2, 2 * K], F16)
            nc.sync.dma_start(out=b2t[:], in_=bias2[:])
            cons_t = constp.tile([P, DCH * K], F16)
            ctiles = [cons_t[:, j * K : (j + 1) * K] for j in range(DCH)]

            warm = warmp.tile([P, K], F16)
            nc.gpsimd.memset(warm[:], 0.0)
            ones2 = warmp.tile([2, P], F16)
            nc.gpsimd.memset(ones2[:], 1.0)
            if table_hoist:
                # first InstActivation triggers the 1.3us ACT_TABLE_LOAD;
                # issue a dummy now so it overlaps the const DMAs instead
                # of gating the first bias copy
                tdum = warmp.tile([P, 1], F32)
                nc.scalar.copy(out=tdum[:], in_=warm[:, :1])

            # PE warmup: dep-free matmuls over a memset SBUF tile into
            # the first pair tiles (overwritten by the bias broadcasts
            # below). They run during the startup DMA wait, ramping the
            # PE p-state so the first real matmuls start at full clock.
            wtiles = [psp.tile([P, 2, HWC], F32, name="ps") for _ in range(4)]
            for w in range(n_warm):
                nc.tensor.matmul(
                    wtiles[(w // 2) % 4][:, w % 2, :K],
                    lhsT=warm[:, :P], rhs=warm[:, :K],
                    start=True, stop=True,
                )
            # extra warms (w>=8) re-warm pairs 0/1 before their bias
            # broadcasts run; ordering is same-tile WAW, handled by Tile

            bft = constp.tile([P, 2, K], F32)

            # sync-ring in-order pipe: group-0 x tiles and consts in
            # exact first-use order
            xch0 = []

            def _load_x0(tl):
                cbuf = xp0.tile([P, DCH, P], F16, name="xtile")
                nc.sync.dma_start(
                    out=cbuf[:], in_=xg0[tl].rearrange("p (j q) -> p j q", j=DCH)
                )
                xch0.append(cbuf)

            _load_x0(0)
            nc.sync.dma_start(out=cons_t[:, : 2 * K], in_=cons[:, : 2 * K])
            _load_x0(1)
            nc.sync.dma_start(out=cons_t[:, 2 * K : 5 * K], in_=cons[:, 2 * K : 5 * K])
            _load_x0(2)
            nc.sync.dma_start(out=cons_t[:, 5 * K :], in_=cons[:, 5 * K :])
            _load_x0(3)

            idxbuf = outp.tile([P, PAIRS + 1, 8], U32)
            tokbuf = outp.tile([P, TILES], I32)
            pending_flush = None

            # schedule: (first_tile, ntiles, src) per group
            sched = [(0, TPG, None), (TPG, 2, xgA[0]), (TPG + 2, 2, xgA[1])]
            for gg in range(GROUPS - 2):
                sched.append((2 * TPG + gg * TPG, TPG, xg[gg]))
            pr = 0
            for t0g, ntl, src in sched:
                if src is None:
                    chunk = lambda j, tl: xch0[tl][:, j, :]
                else:
                    # all steady-state x loads share the sync ring:
                    # same-ring transfers serialize per DMA-engine FIFO,
                    # so prefetch can't steal bandwidth from earlier
                    # (more urgent) loads
                    xbuf = xp.tile([P, DCH, ntl, P], F16, name=f"xgrp{ntl}")
                    nc.sync.dma_start(
                        out=xbuf[:],
                        in_=src.rearrange("p (j t q) -> p j t q", j=DCH, t=ntl),
                    )
                    chunk = lambda j, tl, xbuf=xbuf: xbuf[:, j, tl, :]
                for pl in range(ntl // 2):
                    flush_now, pending_flush = pending_flush, None
                    if pr == PAIRS - 1:
                        if flush_now is not None:
                            _emit_flush(nc, out, tokbuf, idxbuf, flush_now)
                        _emit_last_pair_singles(
                            nc, psp, mxp, chunk, ctiles, bft, out, tokbuf,
                            idxbuf,
                        )
                        continue
                    if pr < 4:
                        # pairs 0-3: bias arrives via two PE broadcast
                        # matmuls (ones2^T @ [bh; bl] accumulates bh+bl
                        # in fp32) - start=True also sets the half-bank's
                        # has_written bits, and the short PE chain beats
                        # waiting for an ACT round-trip at startup
                        ps = wtiles[pr]
                        for h in range(2):
                            nc.tensor.matmul(
                                ps[:, h, :K],
                                lhsT=ones2[:], rhs=b2t[:, h * K : (h + 1) * K],
                                start=True, stop=True,
                            )
                        if pr == 2:
                            # snapshot the bias into SBUF for pairs 4+
                            # (before this pair's matmuls clobber it)
                            nc.scalar.copy(out=bft[:], in_=ps[:, :, :K])
                    else:
                        ps = psp.tile([P, 2, HWC], F32, name="ps")
                        # has_written bits persist from this bank's prior
                        # occupant; ScalarE resets the values to the bias
                        # and the start=False matmuls accumulate on top
                        nc.scalar.copy(out=ps[:, :, :K], in_=bft[:])
                    mxt = mxp.tile([P, 2, 8], F32)
                    for half in range(2):
                        tl = 2 * pl + half
                        assert t0g + tl == 2 * pr + half
                        for j in range(DCH):
                            nc.tensor.matmul(
                                ps[:, half, :K],
                                lhsT=chunk(j, tl),
                                rhs=ctiles[j][:],
                                start=False,
                                stop=(j == DCH - 1),
                                skip_group_check=True,
                            )
                        nc.vector.max(out=mxt[:, half, :], in_=ps[:, half, :K])
                    # Pool packs the FIND keys [A0..A3, B0..B3]; only
                    # lanes 0 (maxA) and 4 (maxB) are consumed, the rest
                    # are benign real values that keep every lane written
                    mx8 = mx8p.tile([P, 8], F32)
                    nc.gpsimd.tensor_copy(out=mx8[:], in_=mxt[:, :, 0:4])
                    # one FIND over the 600-elem pair: out[0] = idx of
                    # maxA (in [0,300)), out[4] = idx of maxB + 300
                    _max_index_nd(
                        nc, out=idxbuf[:, pr, :], in_max=mx8[:],
                        in_values=ps[:, :, :K],
                    )
                    if flush_now is not None:
                        _emit_flush(nc, out, tokbuf, idxbuf, flush_now)
                    # queue this pair's flush for emission one pair later:
                    # emitting it here would park the token-DMA issue in the
                    # scalar queue ahead of the NEXT pair's bias ACTIVATE,
                    # and the in-order queue then stalls the PE on FIND
                    t = 2 * pr + 1
                    if (t + 1) in FLUSH:
                        s = FLUSH[FLUSH.index(t + 1) - 1] if (t + 1) != FLUSH[0] else 0
                        if t + 1 == TILES:
                            _emit_flush(nc, out, tokbuf, idxbuf, (s, t + 1))
                        else:
                            pending_flush = (s, t + 1)
                    pr += 1

    _hoist_excess_waits(nc)
    return nc


def _hoist_excess_waits(nc: bass.Bass, max_waits: int = 1):
    """Hoist excess sync waits onto no-op drains inserted just before.

    Walrus's codegen caps embedded sync waits per instruction (1 for
    DIRECT2D DMAs and CTRL ops), but Tile can attach several (slot-reuse
    WAR + lane WAW, or the kernel-tail drain waiting on every proc).
    A same-engine drain immediately before the instruction blocks the
    sequencer at the same program point, so semantics are unchanged.
    """
    n = 0
    for f in nc.m.functions:
        for blk in f.blocks:
            insts = blk.instructions
            i = 0
            while i < len(insts):
                inst = insts[i]
                si = inst.sync_info
                if si and si.on_wait and len(si.on_wait) > max_waits:
                    waits = list(si.on_wait)
                    si.on_wait = waits[-max_waits:]
                    inst.sync_info = si
                    pre = []
                    for j in range(0, len(waits) - max_waits, max_waits):
                        nd = mybir.InstNoOp(name=f"I-wsplit{n}", ins=[], outs=[])
                        n += 1
                        nd.engine = inst.engine
                        nsi = type(si)(
                            on_wait=waits[j : j + max_waits], on_update=[]
                        )
                        nd.sync_info = nsi
                        try:
                            nc.register_instruction(nd, overwrite=True)
                        except Exception:
                            pass
                        pre.append(nd)
                    for k, nd in enumerate(pre):
                        insts.insert(i + k, nd)
                    i += len(pre)
                i += 1


def make_in_maps(x: np.ndarray, C: np.ndarray, Cnorm: np.ndarray):
    x16 = x.astype(np.float16)
    C16 = C.astype(np.float16).reshape(DCH, P, K)

    cons = np.ascontiguousarray(C16.transpose(1, 0, 2).reshape(P, DCH * K))
    b1 = (-0.5 * Cnorm.reshape(K)).astype(np.float32)
    bh = b1.astype(np.float16)
    bl = (b1 - bh.astype(np.float32)).astype(np.float16)
    bias2 = np.stack([np.concatenate([bh, bh]), np.concatenate([bl, bl])])

    in_maps = []
    for c in range(NCORES):
        xs = x16[c * ROWS : (c + 1) * ROWS]
        # row r = p*128 + g*TPG + tl ; col = j*128 + pd
        xr = xs.reshape(P, GROUPS, TPG, DCH, P)          # [p, g, tl, j, pd]
        xgc = np.ascontiguousarray(xr[:, 2:].transpose(1, 4, 3, 2, 0))  # [g, pd, j, tl, p]
        xg0 = np.ascontiguousarray(xr[:, 0].transpose(1, 3, 2, 0))  # [tl, pd, j, p]
        xgA = np.ascontiguousarray(
            xr[:, 1].reshape(P, 2, 2, DCH, P).transpose(1, 4, 3, 2, 0)
        )  # [half, pd, j, tl2, p]
        in_maps.append(
            {
                "xg": xgc.reshape(GROUPS - 2, P, DCH * TPG * P),
                "xg0": xg0.reshape(TPG, P, DCH * P),
                "xgA": xgA.reshape(2, P, DCH * 2 * P),
                "cons": cons,
                "bias2": bias2,
            }
        )
    return in_maps


_NC_CACHE = {}


def kernel(x, C, Cnorm, b, t):
    global LAST_RESULT
    x = np.asarray(x)
    C = np.asarray(C)
    Cnorm = np.asarray(Cnorm)

    key = 0
    if key not in _NC_CACHE:
        _NC_CACHE[key] = build_nc()
    nc = _NC_CACHE[key]

    in_maps = make_in_maps(x, C, Cnorm)
    trace = bool(int(os.environ.get("KM_TRACE", "0")))
    if trace:
        _ensure_ntff_hook()
    res = run_bass_kernel_spmd(
        nc, in_maps, core_ids=list(range(NCORES)), trace=trace
    )
    LAST_RESULT = res

    shards = [res.results[c]["out"].reshape(-1) for c in range(NCORES)]
    tokens = np.concatenate(shards).astype(np.int32)
    return tokens.reshape(int(b), int(t))
